# revision 19
# baseline (speedup 1.0000x reference)
# GATv2 encoder (3x GATv2Conv, H=1) on 8 Trainium2 NeuronCores.
#
# Sharding: nodes partitioned by dst across 8 cores (graph parallel).
# Edge work per core is organized as 98 "bins" of <=128 dst nodes each
# (host-side bin-packing balances edge counts); a bin's edges are grouped
# by source-table chunk (4 chunks of 25088 rows so indices fit int16) and
# padded to 128-edge tiles.  Per bin: batched dma_gather of att-scaled
# source rows xl'[src] (trailing -1 indices skip pad descriptors), xr[dst]
# reconstructed on the TensorEngine from the SBUF-resident local XR table
# via a host-baked one-hot (OT), leaky-relu logits on the ScalarEngine
# (att folded into the tables; columns permuted by sign(att) host-side),
# and per-tile one-hot matmuls accumulating softmax numerator + denominator
# in one PSUM bank.  Host gathers per-core dense outputs between launches.
import os
import sys
import math
import functools
import numpy as np

for _p in ("/opt/trn_rl_repo",):
    if _p not in sys.path and os.path.isdir(_p):
        sys.path.insert(0, _p)

import concourse.bass as bass
import concourse.mybir as mybir
import concourse.tile as tile
from concourse import bacc

F32 = mybir.dt.float32
BF16 = mybir.dt.bfloat16
I16 = mybir.dt.int16
I32 = mybir.dt.int32
AF = mybir.ActivationFunctionType
ALU = mybir.AluOpType

# Problem constants (hardcoded per contract)
N = 100_000
E = 1_600_000
IN, HID, OUT, H = 256, 128, 64, 1
SLOPE = 0.2
NCORES = 8
P = 128
EPS = 1e-30
ECLAMP = 30.0       # logit clamp: keeps exp() finite on stale pad slots
NCHUNK = 4          # source-table chunks (rows per chunk must fit int16)


class Cfg:
    """Geometry, parameterized so small test instances can be built."""

    def __init__(self, n=N, e=E, fin=IN, hid=HID, out=OUT, ncores=NCORES):
        self.n, self.e, self.fin, self.hid, self.out = n, e, fin, hid, out
        self.ncores = ncores
        assert n % ncores == 0
        self.nl = n // ncores                  # dst nodes per core
        self.nbins = math.ceil(self.nl / P)    # bins per core
        self.nlp = self.nbins * P              # padded local nodes
        self.ntab = self.nlp * ncores          # rows in gathered tables
        self.nchunk = min(NCHUNK, ncores)
        assert self.ntab % self.nchunk == 0
        self.chrows = self.ntab // self.nchunk  # rows per source chunk
        assert self.chrows < 32768, "chunk rows must fit int16"
        assert fin % P == 0
        self.kt = fin // P                     # K-tiles for dense1


# ----------------------------------------------------------------------------
# Host-side graph preprocessing
# ----------------------------------------------------------------------------

def prep_graph(cfg: Cfg, edge_index: np.ndarray):
    """Bin-pack dsts, group edges by (bin, src chunk), build index arrays."""
    n, ncores, nl, nbins, nlp = cfg.n, cfg.ncores, cfg.nl, cfg.nbins, cfg.nlp
    nck, chrows = cfg.nchunk, cfg.chrows
    src = np.concatenate([edge_index[0], np.arange(n, dtype=np.int64)])
    dst = np.concatenate([edge_index[1], np.arange(n, dtype=np.int64)])

    # --- per-core bin-packing of dst nodes ---------------------------------
    slot_global = np.full(n, -1, dtype=np.int64)  # node -> row in table space
    deg_all = np.bincount(dst, minlength=n)
    import heapq
    for c in range(ncores):
        lo, hi = c * nl, (c + 1) * nl
        deg = deg_all[lo:hi]
        order = np.argsort(-deg, kind="stable")
        heap = [(0, 0, b) for b in range(nbins)]
        heapq.heapify(heap)
        stash = []
        for node in order:
            d = int(deg[node])
            while True:
                s, cnt, b = heapq.heappop(heap)
                if cnt < P:
                    break
                stash.append((s, cnt, b))
            slot_global[lo + node] = c * nlp + b * P + cnt
            heapq.heappush(heap, (s + d, cnt + 1, b))
            for it in stash:
                heapq.heappush(heap, it)
            stash.clear()

    # --- group edges by (core, bin, chunk) ---------------------------------
    sslot = slot_global[src]
    dslot = slot_global[dst]
    chunk = sslot // chrows
    binid = dslot // P                    # global bin id = core*nbins + bin
    key = binid * nck + chunk
    order = np.argsort(key, kind="stable")
    s_o, d_o, k_o = sslot[order], dslot[order], key[order]
    nkeys = ncores * nbins * nck
    cnts = np.bincount(k_o, minlength=nkeys).reshape(ncores, nbins, nck)
    offs = np.concatenate([[0], np.cumsum(cnts.reshape(-1))])

    # uniform-across-cores tiles per (bin, chunk)
    tbo = np.maximum(np.ceil(cnts / P).astype(np.int64).max(axis=0),
                     (cnts.max(axis=0) > 0))  # [nbins, nck]
    tbin = tbo.sum(axis=1)                 # [nbins] tiles per bin
    nslots = int(tbin.sum()) * P           # edge slots per core
    # uniform valid-index count per (bin, chunk): max across cores; slots
    # beyond it carry -1 indices, which the gather ucode skips (trailing
    # negatives emit no DMA descriptors).
    vcnt = np.maximum(cnts.max(axis=0), (tbo > 0)).astype(np.int64)

    # --- per-core index arrays ---------------------------------------------
    # gidx16: wrapped-16 int16 chunk-local src indices, [128, nslots//16]
    # dstcol: lane-major one-hot columns (f32),         [128, nslots//128]
    # onehotT: OT[d, slot] = 1 iff slot's dst row (bin-local) == d
    gidx16 = np.zeros((ncores, 128, nslots // 16), np.int16)
    dstcol = np.full((ncores, 128, nslots // 128), 200.0, np.float32)
    onehotT = np.zeros((ncores, 128, nslots), np.float32)
    for c in range(ncores):
        pos = 0  # slot position within the core's stream
        for b in range(nbins):
            rbase = b * P
            for o in range(nck):
                kk = int(cnts[c, b, o])
                so = offs[(c * nbins + b) * nck + o]
                slots = int(tbo[b, o]) * P
                if slots == 0:
                    continue
                j = np.arange(kk)
                jp = pos + j
                vc = int(vcnt[b, o])
                # [0:kk] real, [kk:vc] index-0 pads (uniform descriptor
                # count across cores), [vc:slots] -1 (skipped by ucode)
                g = np.full(slots, -1, np.int16)
                g[j] = (s_o[so:so + kk] - o * chrows).astype(np.int16)
                g[kk:vc] = 0
                jj = pos + np.arange(slots)
                gidx16[c, jj % 16, jj // 16] = g
                dloc = (d_o[so:so + kk] - (c * nlp + rbase)).astype(np.int64)
                dstcol[c, jp % 128, jp // 128] = dloc.astype(np.float32)
                onehotT[c, dloc, jp] = 1.0
                pos += slots
        assert pos == nslots
        # the Q7 gather ucode reads indices from its own 16-partition group:
        # replicate the wrapped-16 data across all 8 groups
        gidx16[c] = np.tile(gidx16[c, :16], (8, 1))

    # node permutation per core: slot s -> original node (or -1)
    perm = np.full((ncores, nlp), -1, dtype=np.int64)
    nodes = np.where(slot_global >= 0)[0]
    perm.reshape(-1)[slot_global[nodes]] = nodes

    return dict(
        tbo=tbo, tbin=tbin, vcnt=vcnt, nslots=nslots,
        slot_global=slot_global, perm=perm,
        gidx16=gidx16, dstcol=dstcol, onehotT=onehotT,
    )


# ----------------------------------------------------------------------------
# Device program builders (single SPMD program, data differs per core)
# ----------------------------------------------------------------------------

def _new_nc(cfg, nq=1):
    return bacc.Bacc("TRN2", target_bir_lowering=False, debug=False,
                     enable_asserts=False, num_devices=cfg.ncores,
                     num_swdge_queues=nq)


def build_dense1(cfg: Cfg, dt=F32):
    """xT [fin, nlp] -> XL1 [nlp, hid], XR1 [nlp, hid]."""
    nc = _new_nc(cfg)
    fin, hid, nlp, kt = cfg.fin, cfg.hid, cfg.nlp, cfg.kt
    xT = nc.dram_tensor("xT", [fin, nlp], F32, kind="ExternalInput")
    wl = nc.dram_tensor("wl", [fin, hid], F32, kind="ExternalInput")
    wr = nc.dram_tensor("wr", [fin, hid], F32, kind="ExternalInput")
    blB = nc.dram_tensor("blB", [P, hid], F32, kind="ExternalInput")
    brB = nc.dram_tensor("brB", [P, hid], F32, kind="ExternalInput")
    XL = nc.dram_tensor("XL1", [nlp, hid], dt, kind="ExternalOutput")
    XR = nc.dram_tensor("XR1", [nlp, hid], dt, kind="ExternalOutput")

    mtiles = nlp // P
    with tile.TileContext(nc) as tc:
        with tc.tile_pool(name="const", bufs=1) as cp, \
             tc.tile_pool(name="work", bufs=4) as wp, \
             tc.tile_pool(name="psum", bufs=4, space="PSUM") as pp:
            xk = cp.tile([P, kt, nlp], F32)
            nc.sync.dma_start(xk[:], xT[:].rearrange("(k p) n -> p k n", p=P))
            wl_sb = cp.tile([P, kt, hid], F32)
            nc.sync.dma_start(wl_sb[:], wl[:].rearrange("(k p) h -> p k h", p=P))
            wr_sb = cp.tile([P, kt, hid], F32)
            nc.sync.dma_start(wr_sb[:], wr[:].rearrange("(k p) h -> p k h", p=P))
            blB_sb = cp.tile([P, hid], F32)
            nc.sync.dma_start(blB_sb[:], blB[:])
            brB_sb = cp.tile([P, hid], F32)
            nc.sync.dma_start(brB_sb[:], brB[:])

            for m in range(mtiles):
                ms = slice(m * P, (m + 1) * P)
                psl = pp.tile([P, hid], F32, tag="psl")
                psr = pp.tile([P, hid], F32, tag="psr")
                for k in range(kt):
                    nc.tensor.matmul(psl[:], lhsT=xk[:, k, ms], rhs=wl_sb[:, k, :],
                                     start=(k == 0), stop=(k == kt - 1))
                for k in range(kt):
                    nc.tensor.matmul(psr[:], lhsT=xk[:, k, ms], rhs=wr_sb[:, k, :],
                                     start=(k == 0), stop=(k == kt - 1))
                ol = wp.tile([P, hid], dt, tag="ol")
                nc.vector.tensor_tensor(out=ol[:], in0=psl[:], in1=blB_sb[:], op=ALU.add)
                orr = wp.tile([P, hid], dt, tag="orr")
                nc.vector.tensor_tensor(out=orr[:], in0=psr[:], in1=brB_sb[:], op=ALU.add)
                nc.sync.dma_start(XL[ms, :], ol[:])
                nc.sync.dma_start(XR[ms, :], orr[:])
    nc.compile()
    return nc


def _edge_phase(nc, tc, cfg, pr, pools, tabs, consts, n_lay, kpos, finalize, dt):
    """Shared edge pipeline over bins (att folded into the tables).

    Gathered rows are 128 wide: n_lay layer blocks of feat=128//n_lay cols;
    within each block, columns with positive att come first (kpos[l] of
    them), as permuted host-side.  xr[dst] is reconstructed per tile on the
    TensorEngine from the SBUF-resident XR table via the host-baked one-hot
    OT.  finalize(b, psums): consume accumulated PSUM tiles per bin
    (cols [0:feat] numerator, col [feat] denominator).
    """
    from concourse import library_config
    nc.gpsimd.load_library(library_config.mlp)
    cp, gp, wp, pp = pools
    XLchunks, XR, GIDX, DCOL, OHT = tabs
    iotaRep_sb = consts
    tbo, tbin, vcnt = pr["tbo"], pr["tbin"], pr["vcnt"]
    nslots = pr["nslots"]
    nck, chrows = cfg.nchunk, cfg.chrows
    feat = P // n_lay
    Tmax = int(tbin.max())
    NQ = nc.num_swdge_queues
    qn = 0

    # whole-launch index array resident in SBUF
    gix = cp.tile([P, nslots // 16], I16)
    nc.sync.dma_start(gix[:], GIDX[:])
    dcl = cp.tile([P, nslots // 128, 1], dt)
    nc.sync.dma_start(dcl[:], DCOL[:])
    # local XR table resident: [p, bin, hid] with p = row within bin
    xrt = cp.tile([P, cfg.nbins, P], dt)
    nc.sync.dma_start(xrt[:], XR[:].rearrange("(b p) h -> p b h", p=P))

    bpos = np.concatenate([[0], np.cumsum(tbin)]) * P
    st = {}

    def stage1(b):
        """Gathers + OT stream + xr reconstruction (PE matmul, ACT copy)."""
        nonlocal qn
        pos = int(bpos[b])
        Tb = int(tbin[b])
        G = gp.tile([P, Tmax, P], dt, tag="G", name="G")
        OT = gp.tile([P, Tmax, P], dt, tag="OT", name="OT")
        Rs = gp.tile([P, Tmax, P], dt, tag="Rs", name="Rs")
        if b < 2:
            # first use of each pool buffer: clear so skipped (-1) pad
            # slots hold finite values, not uninitialized SBUF bits
            nc.vector.memset(G[:], 0.0)
        to = 0
        for o in range(nck):
            tt = int(tbo[b, o])
            if tt == 0:
                continue
            nidx = tt * P
            vc = int(vcnt[b, o])
            col = (pos + to * P) // 16
            nc.gpsimd.dma_gather(
                out_ap=G[:, to:to + tt, :],
                in_ap=XLchunks[o][:],
                idxs_ap=gix[:, col:col + nidx // 16],
                num_idxs=nidx, num_idxs_reg=vc, elem_size=P,
                single_packet=(nidx <= 1024), queue_num=qn % NQ)
            qn += 1
            to += tt
        assert to == Tb
        nc.sync.dma_start(OT[:, 0:Tb, :],
                          OHT[:, pos:pos + Tb * P].rearrange(
                              "d (t e) -> d t e", e=P))
        # xr[dst] per edge: R_t = OT_t^T @ XR_bin on the TensorEngine.
        # 4 tiles share one PSUM bank (slice 0's start=True zeroes the whole
        # bank; later slices accumulate onto zeroed regions), so one ACT
        # copy moves 4 tiles to SBUF (bf16) and the z-add runs 2x batched.
        for t0 in range(0, Tb, 4):
            gsz = min(4, Tb - t0)
            pR = pp.tile([P, 4, P], F32, tag="pR", name="pR")
            for i in range(gsz):
                nc.tensor.matmul(pR[:, i, :], lhsT=OT[:, t0 + i, :],
                                 rhs=xrt[:, b, :],
                                 start=(i == 0), stop=(i == gsz - 1))
            nc.scalar.activation(out=Rs[:, t0:t0 + gsz, :],
                                 in_=pR[:, 0:gsz, :], func=AF.Copy)
        st[b] = dict(G=G, OT=OT, Rs=Rs, pos=pos, Tb=Tb)

    def stage2(b):
        """Logits + softmax weights (DVE + ACT)."""
        s = st[b]
        G, Rs, pos, Tb = s["G"], s["Rs"], s["pos"], s["Tb"]
        dcol = dcl[:, pos // P:(pos + Tb * P) // P, :]   # [P, Tb, 1]
        # z = xl'[src] + xr'[dst]  (att-scaled space)
        Z = wp.tile([P, Tmax, P], dt, tag="Z", name="Z")
        nc.vector.tensor_tensor(out=Z[:, 0:Tb, :], in0=G[:, 0:Tb, :],
                                in1=Rs[:, 0:Tb, :], op=ALU.add)
        # signed-att leaky-relu: positive-att cols get lrelu(z) = max(z,.2z),
        # negative-att cols get min(z,.2z) = Prelu(alpha=1/s)(s*z), s=.2 --
        # then the plain row-sum per layer IS the logit.
        U = wp.tile([P, Tmax, P], dt, tag="U", name="U")
        for l in range(n_lay):
            k = kpos[l]
            c0 = l * feat
            if k > 0:
                nc.scalar.activation(out=U[:, 0:Tb, c0:c0 + k],
                                     in_=Z[:, 0:Tb, c0:c0 + k],
                                     func=AF.Prelu, alpha=SLOPE)
            if k < feat:
                nc.scalar.activation(out=U[:, 0:Tb, c0 + k:c0 + feat],
                                     in_=Z[:, 0:Tb, c0 + k:c0 + feat],
                                     func=AF.Prelu, alpha=1.0 / SLOPE,
                                     scale=SLOPE)
        e = wp.tile([P, Tmax, n_lay, 1], F32, tag="e", name="e")
        nc.vector.tensor_reduce(
            out=e[:, 0:Tb, :, :],
            in_=U[:, 0:Tb, :].rearrange("p t (l f) -> p t l f", l=n_lay),
            axis=mybir.AxisListType.X, op=ALU.add)
        # clamp: skipped pad slots carry stale-but-finite data whose logit
        # can be large; exp must stay finite so 0*exp(e) == 0 in the matmul
        nc.vector.tensor_scalar(out=e[:, 0:Tb, :, :], in0=e[:, 0:Tb, :, :],
                                scalar1=ECLAMP, scalar2=None, op0=ALU.min)
        ee = wp.tile([P, Tmax, n_lay, 1], dt, tag="ee", name="ee")
        nc.scalar.activation(out=ee[:, 0:Tb, :, :], in_=e[:, 0:Tb, :, :],
                             func=AF.Exp)

        # one-hot (dst-col) and attention-weighted one-hot
        O = wp.tile([P, Tmax, 1, P], dt, tag="O", name="O")
        nc.vector.tensor_tensor(
            out=O[:, 0:Tb, 0, :],
            in0=dcol.to_broadcast([P, Tb, P]),
            in1=iotaRep_sb[:, 0:Tb, :], op=ALU.is_equal)
        A = wp.tile([P, Tmax, n_lay, P], dt, tag="A", name="A")
        nc.vector.tensor_tensor(
            out=A[:, 0:Tb, :, :],
            in0=O[:, 0:Tb, :, :].to_broadcast([P, Tb, n_lay, P]),
            in1=ee[:, 0:Tb, :, :].to_broadcast([P, Tb, n_lay, P]),
            op=ALU.mult)
        s.update(O=O, A=A, ee=ee)

    def stage3(b):
        """Accumulate numerator/denominator (PE) and finalize the bin."""
        s = st.pop(b)
        G, O, A, ee, Tb = s["G"], s["O"], s["A"], s["ee"], s["Tb"]
        # numerator cols [0:feat], denominator col [feat]: one PSUM bank per
        # layer.  num's start=True zeroes the whole bank, so the den matmul
        # always runs start=False and lands on a zeroed column.
        pn = [pp.tile([P, feat + n_lay if l == 0 else feat], F32,
                      tag=f"pn{l}", name=f"pn{l}")
              for l in range(n_lay)]
        for t in range(Tb):
            for l in range(n_lay):
                nc.tensor.matmul(
                    pn[l][:, 0:feat], lhsT=A[:, t, l, :],
                    rhs=G[:, t, l * feat:(l + 1) * feat],
                    start=(t == 0), stop=(t == Tb - 1))
            nc.tensor.matmul(
                pn[0][:, feat:feat + n_lay], lhsT=O[:, t, 0, :],
                rhs=ee[:, t, :, 0], start=False, stop=(t == Tb - 1))
        finalize(b, pn)

    # 1-bin software pipeline: keep the TensorEngine fed (its stream is
    # in-order, so bin b+1's xr matmuls must be emitted before bin b's
    # accumulation matmuls to overlap with the DVE/ACT middle stages).
    stage1(0)
    for b in range(cfg.nbins):
        if b + 1 < cfg.nbins:
            stage1(b + 1)
        stage2(b)
        stage3(b)


def _load_consts(nc, cp, names_shapes):
    out = []
    for name, shape, dt in names_shapes:
        dram = nc.dram_tensor(name, shape, dt, kind="ExternalInput")
        sb = cp.tile(shape, dt, name=name + "_sb")
        nc.sync.dma_start(sb[:], dram[:])
        out.append(sb)
    return out


def build_edge1(cfg: Cfg, pr, kpos, dt=F32, nq=1):
    """Edge phase of layer 1 + dense transforms of layers 2/3."""
    nc = _new_nc(cfg, nq)
    hid, out, nlp, nbins = cfg.hid, cfg.out, cfg.nlp, cfg.nbins
    nslots = pr["nslots"]
    Tmax = int(pr["tbin"].max())
    XLchunks = [nc.dram_tensor(f"XL1c{o}", [cfg.chrows, hid], dt,
                               kind="ExternalInput") for o in range(cfg.nchunk)]
    XR = nc.dram_tensor("XR1", [nlp, hid], dt, kind="ExternalInput")
    GIDX = nc.dram_tensor("gidx", [P, nslots // 16], I16, kind="ExternalInput")
    DCOL = nc.dram_tensor("dcol", [P, nslots // 128, 1], dt, kind="ExternalInput")
    OHT = nc.dram_tensor("oht", [P, nslots], dt, kind="ExternalInput")
    XL23 = nc.dram_tensor("XL23", [nlp, P], dt, kind="ExternalOutput")
    XR23 = nc.dram_tensor("XR23", [nlp, P], dt, kind="ExternalOutput")

    with tile.TileContext(nc) as tc:
        with tc.tile_pool(name="const", bufs=1) as cp, \
             tc.tile_pool(name="gath", bufs=2) as gp, \
             tc.tile_pool(name="work", bufs=2) as wp, \
             tc.tile_pool(name="psum", bufs=2, space="PSUM") as pp, \
             tc.tile_pool(name="psfin", bufs=2, space="PSUM") as pf:
            (iotaRep_sb, winv1B_sb, b1B_sb, ident_sb, w23lr_sb,
             b23lr_sb) = _load_consts(nc, cp, [
                 ("iotaRep", [P, Tmax, P], dt),
                 ("winv1B", [P, hid], F32),
                 ("b1B", [P, hid], F32),
                 ("identB", [P, P], F32),
                 ("w23lr", [hid, 2 * P], F32),
                 ("b23lrB", [P, 2 * P], F32)])

            def finalize(b, pn):
                ms = slice(b * P, (b + 1) * P)
                d = wp.tile([P, 1], F32, tag="d", name="d")
                nc.vector.tensor_scalar_add(d[:], pn[0][:, hid:hid + 1], EPS)
                r = wp.tile([P, 1], F32, tag="r", name="r")
                nc.vector.reciprocal(r[:], d[:])
                h = wp.tile([P, hid], F32, tag="h", name="h")
                # h = (num * 1/den) * winv  (unscale att, permuted order)
                nc.vector.scalar_tensor_tensor(
                    out=h[:], in0=pn[0][:, 0:hid], scalar=r[:],
                    in1=winv1B_sb[:], op0=ALU.mult, op1=ALU.mult)
                nc.vector.tensor_tensor(out=h[:], in0=h[:], in1=b1B_sb[:], op=ALU.add)
                nc.scalar.activation(out=h[:], in_=h[:], func=AF.Relu)
                pst = pf.tile([P, P], F32, tag="pst", name="pst")
                nc.tensor.transpose(out=pst[:], in_=h[:], identity=ident_sb[:])
                hT = wp.tile([P, P], F32, tag="hT", name="hT")
                nc.scalar.activation(out=hT[:], in_=pst[:], func=AF.Copy)
                psB = pf.tile([P, 2 * P], F32, tag="psB", name="psB")
                nc.tensor.matmul(psB[:], lhsT=hT[:, 0:hid], rhs=w23lr_sb[:],
                                 start=True, stop=True)
                olr = wp.tile([P, 2 * P], dt, tag="olr", name="olr")
                nc.vector.tensor_tensor(out=olr[:], in0=psB[:], in1=b23lr_sb[:],
                                        op=ALU.add)
                nc.sync.dma_start(XL23[ms, :], olr[:, 0:P])
                nc.sync.dma_start(XR23[ms, :], olr[:, P:2 * P])

            _edge_phase(nc, tc, cfg, pr, (cp, gp, wp, pp),
                        (XLchunks, XR, GIDX, DCOL, OHT),
                        iotaRep_sb, 1, kpos, finalize, dt)
    nc.compile()
    return nc


def build_edge23(cfg: Cfg, pr, kpos, dt=F32, nq=1):
    """Edge phases of layers 2 and 3 (shared gather)."""
    nc = _new_nc(cfg, nq)
    out, nlp, nbins = cfg.out, cfg.nlp, cfg.nbins
    nslots = pr["nslots"]
    Tmax = int(pr["tbin"].max())
    XLchunks = [nc.dram_tensor(f"XL23c{o}", [cfg.chrows, P], dt,
                               kind="ExternalInput") for o in range(cfg.nchunk)]
    XR = nc.dram_tensor("XR23", [nlp, P], dt, kind="ExternalInput")
    GIDX = nc.dram_tensor("gidx", [P, nslots // 16], I16, kind="ExternalInput")
    DCOL = nc.dram_tensor("dcol", [P, nslots // 128, 1], dt, kind="ExternalInput")
    OHT = nc.dram_tensor("oht", [P, nslots], dt, kind="ExternalInput")
    MU = nc.dram_tensor("MU", [nlp, out], F32, kind="ExternalOutput")
    LV = nc.dram_tensor("LV", [nlp, out], F32, kind="ExternalOutput")

    with tile.TileContext(nc) as tc:
        with tc.tile_pool(name="const", bufs=1) as cp, \
             tc.tile_pool(name="gath", bufs=2) as gp, \
             tc.tile_pool(name="work", bufs=2) as wp, \
             tc.tile_pool(name="psum", bufs=2, space="PSUM") as pp:
            (iotaRep_sb, winvmuB_sb, winvlvB_sb, bmu_sb, blv_sb) = _load_consts(
                nc, cp, [
                    ("iotaRep", [P, Tmax, P], dt),
                    ("winvmuB", [P, out], F32),
                    ("winvlvB", [P, out], F32),
                    ("bmuB", [P, out], F32),
                    ("blvB", [P, out], F32)])

            def finalize(b, pn):
                ms = slice(b * P, (b + 1) * P)
                for l, (winv, bias, dest, tg) in enumerate(
                        ((winvmuB_sb, bmu_sb, MU, "mu"),
                         (winvlvB_sb, blv_sb, LV, "lv"))):
                    d = wp.tile([P, 1], F32, tag=f"d{tg}", name="d")
                    nc.vector.tensor_scalar_add(d[:], pn[0][:, out + l:out + l + 1],
                                                EPS)
                    r = wp.tile([P, 1], F32, tag=f"r{tg}", name="r")
                    nc.vector.reciprocal(r[:], d[:])
                    o = wp.tile([P, out], F32, tag=f"o{tg}", name="o")
                    nc.vector.scalar_tensor_tensor(
                        out=o[:], in0=pn[l][:, 0:out], scalar=r[:],
                        in1=winv[:], op0=ALU.mult, op1=ALU.mult)
                    nc.vector.tensor_tensor(out=o[:], in0=o[:], in1=bias[:], op=ALU.add)
                    nc.sync.dma_start(dest[ms, :], o[:])

            _edge_phase(nc, tc, cfg, pr, (cp, gp, wp, pp),
                        (XLchunks, XR, GIDX, DCOL, OHT),
                        iotaRep_sb, 2, kpos, finalize, dt)
    nc.compile()
    return nc


# ----------------------------------------------------------------------------
# Host orchestration
# ----------------------------------------------------------------------------

def _bb(v, rows=P):
    """Broadcast a 1-D row vector to [rows, len] f32."""
    v = np.asarray(v, np.float32).reshape(1, -1)
    return np.ascontiguousarray(np.broadcast_to(v, (rows, v.shape[1])))


def _hw_runner(nc, in_maps, cfg, trace=False):
    from concourse import bass_utils
    r = bass_utils.run_bass_kernel_spmd(
        nc, in_maps, core_ids=list(range(cfg.ncores)), trace=trace)
    return r.results, r.exec_time_ns


class _State:
    """Cached compiled programs + prep, keyed by edge structure."""
    key = None
    progs = None
    prep = None
    fold = None


EDT = BF16 if not int(os.environ.get("GAT_F32", "0")) else F32
NQUEUES = 4


def fold_weights(w):
    """Fold att into the tables: scale columns by att (signed), permute so
    positive-att columns come first in each layer block.  Returns folded
    weights + the inverse data needed at finalize / host postprocess."""
    f = {}
    perms, kpos = {}, {}
    for name, fo in (("sh", HID), ("mu", OUT), ("lv", OUT)):
        a = np.asarray(w[f"{name}_att"], np.float32).reshape(-1)  # [fo]
        pi = np.argsort(a <= 0, kind="stable")  # positive cols first
        perms[name] = pi
        kpos[name] = int((a > 0).sum())
        f[f"{name}_a"] = np.ascontiguousarray(a[pi])  # signed att, permuted
        f[f"{name}_Wl"] = np.ascontiguousarray(
            (np.asarray(w[f"{name}_Wl"], np.float32) * a)[:, pi])
        f[f"{name}_Wr"] = np.ascontiguousarray(
            (np.asarray(w[f"{name}_Wr"], np.float32) * a)[:, pi])
        f[f"{name}_bl"] = (np.asarray(w[f"{name}_bl"], np.float32) * a)[pi]
        f[f"{name}_br"] = (np.asarray(w[f"{name}_br"], np.float32) * a)[pi]
        f[f"{name}_winv"] = 1.0 / f[f"{name}_a"]
        f[f"{name}_b"] = np.asarray(w[f"{name}_b"], np.float32)[pi]
    # layer-1 permutation also permutes h's columns -> permute W23 rows
    pi1 = perms["sh"]
    f["mu_Wl"] = np.ascontiguousarray(f["mu_Wl"][pi1])
    f["mu_Wr"] = np.ascontiguousarray(f["mu_Wr"][pi1])
    f["lv_Wl"] = np.ascontiguousarray(f["lv_Wl"][pi1])
    f["lv_Wr"] = np.ascontiguousarray(f["lv_Wr"][pi1])
    f["perms"] = perms
    f["kpos"] = kpos
    return f


def build_progs(cfg, pr, fold, dt=None, nq=None):
    dt = EDT if dt is None else dt
    nq = NQUEUES if nq is None else nq
    kp = fold["kpos"]
    return dict(
        dense1=build_dense1(cfg, dt),
        edge1=build_edge1(cfg, pr, [kp["sh"]], dt, nq),
        edge23=build_edge23(cfg, pr, [kp["mu"], kp["lv"]], dt, nq),
    )


def forward(cfg, x, ei_unused, w, f, pr, progs, runner, dt=None):
    dt = EDT if dt is None else dt
    ndt = mybir.dt.np(dt)
    perm = pr["perm"]                    # [ncores, nlp] node ids or -1
    Tmax = int(pr["tbin"].max())
    profile = {}
    dcol_in = [np.ascontiguousarray(pr["dstcol"][c][:, :, None].astype(ndt))
               for c in range(cfg.ncores)]
    oht_in = [np.ascontiguousarray(pr["onehotT"][c].astype(ndt))
              for c in range(cfg.ncores)]

    hid, out, nlp, ntab = cfg.hid, cfg.out, cfg.nlp, cfg.ntab

    # ---- launch A: dense1 (att-scaled, sign-permuted weights) -------------
    in_maps = []
    for c in range(cfg.ncores):
        xs = np.zeros((nlp, cfg.fin), np.float32)
        sel = perm[c] >= 0
        xs[sel] = x[perm[c][sel]]
        in_maps.append(dict(
            xT=np.ascontiguousarray(xs.T), wl=f["sh_Wl"], wr=f["sh_Wr"],
            blB=_bb(f["sh_bl"]), brB=_bb(f["sh_br"])))
    rA, profile["A"] = runner(progs["dense1"], in_maps, cfg)
    XL1full = np.concatenate([rA[c]["XL1"] for c in range(cfg.ncores)])
    XL1ch = {f"XL1c{o}": np.ascontiguousarray(
        XL1full[o * cfg.chrows:(o + 1) * cfg.chrows])
        for o in range(cfg.nchunk)}
    XR1 = [rA[c]["XR1"] for c in range(cfg.ncores)]

    # ---- launch B: edge1 + dense23 ----------------------------------------
    iotaRep = np.ascontiguousarray(np.broadcast_to(
        np.arange(P, dtype=np.float32), (P, Tmax, P))).astype(ndt)
    w23l = np.concatenate([f["mu_Wl"], f["lv_Wl"]], axis=1)
    w23r = np.concatenate([f["mu_Wr"], f["lv_Wr"]], axis=1)
    b23l = np.concatenate([f["mu_bl"], f["lv_bl"]])
    b23r = np.concatenate([f["mu_br"], f["lv_br"]])
    ident = np.eye(P, dtype=np.float32)
    in_maps = []
    for c in range(cfg.ncores):
        in_maps.append(dict(
            XR1=XR1[c], **XL1ch,
            gidx=pr["gidx16"][c], dcol=dcol_in[c], oht=oht_in[c],
            iotaRep=iotaRep, winv1B=_bb(f["sh_winv"]), b1B=_bb(f["sh_b"]),
            identB=ident,
            w23lr=np.ascontiguousarray(np.concatenate([w23l, w23r], axis=1)),
            b23lrB=_bb(np.concatenate([b23l, b23r]))))
    rB, profile["B"] = runner(progs["edge1"], in_maps, cfg)
    XL23full = np.concatenate([rB[c]["XL23"] for c in range(cfg.ncores)])
    XL23ch = {f"XL23c{o}": np.ascontiguousarray(
        XL23full[o * cfg.chrows:(o + 1) * cfg.chrows])
        for o in range(cfg.nchunk)}
    XR23 = [rB[c]["XR23"] for c in range(cfg.ncores)]

    # ---- launch C: edge23 --------------------------------------------------
    in_maps = []
    for c in range(cfg.ncores):
        in_maps.append(dict(
            XR23=XR23[c], **XL23ch,
            gidx=pr["gidx16"][c], dcol=dcol_in[c], oht=oht_in[c],
            iotaRep=iotaRep,
            winvmuB=_bb(f["mu_winv"]), winvlvB=_bb(f["lv_winv"]),
            bmuB=_bb(f["mu_b"]), blvB=_bb(f["lv_b"])))
    rC, profile["C"] = runner(progs["edge23"], in_maps, cfg)

    MU = np.concatenate([rC[c]["MU"] for c in range(cfg.ncores)])
    LV = np.concatenate([rC[c]["LV"] for c in range(cfg.ncores)])
    # undo the sign permutation of the output columns
    mu_u = np.empty_like(MU)
    mu_u[:, f["perms"]["mu"]] = MU
    lv_u = np.empty_like(LV)
    lv_u[:, f["perms"]["lv"]] = LV
    mu = mu_u[pr["slot_global"]]
    lv = lv_u[pr["slot_global"]]
    return (mu, lv), profile


def kernel(**inputs):
    cfg = Cfg()
    x = np.asarray(inputs["x"], np.float32)
    ei = np.asarray(inputs["edge_index"]).astype(np.int64)
    w = {k: np.asarray(v, np.float32) for k, v in inputs.items()
         if k not in ("x", "edge_index")}

    fold = fold_weights(w)
    key = (hash(ei.tobytes()), tuple(sorted(fold["kpos"].items())))
    if _State.key != key:
        pr = prep_graph(cfg, ei)
        _State.prep = pr
        _State.progs = build_progs(cfg, pr, fold)
        _State.key = key
    _State.fold = fold

    trace = bool(int(os.environ.get("GAT_TRACE", "0")))
    runner = functools.partial(_hw_runner, trace=trace)
    (mu, lv), profile = forward(cfg, x, ei, w, _State.fold, _State.prep,
                                _State.progs, runner)
    kernel._last_profile = profile
    return (mu, lv)


kernel._last_profile = None


# revision 20
# speedup vs baseline: 1.0844x; 1.0844x over previous
# GATv2 encoder (3x GATv2Conv, H=1) on 8 Trainium2 NeuronCores.
#
# Sharding: nodes partitioned by dst across 8 cores (graph parallel).
# Edge work per core is organized as 98 "bins" of <=128 dst nodes each
# (host-side bin-packing balances edge counts); a bin's edges are grouped
# by source-table chunk (4 chunks of 25088 rows so indices fit int16) and
# padded to 128-edge tiles.  Per bin: batched dma_gather of att-scaled
# source rows xl'[src] (trailing -1 indices skip pad descriptors), xr[dst]
# reconstructed on the TensorEngine from the SBUF-resident local XR table
# via a host-baked one-hot (OT), leaky-relu logits on the ScalarEngine
# (att folded into the tables; columns permuted by sign(att) host-side),
# and per-tile one-hot matmuls accumulating softmax numerator + denominator
# in one PSUM bank.  Host gathers per-core dense outputs between launches.
import os
import sys
import math
import functools
import numpy as np

for _p in ("/opt/trn_rl_repo",):
    if _p not in sys.path and os.path.isdir(_p):
        sys.path.insert(0, _p)

import concourse.bass as bass
import concourse.mybir as mybir
import concourse.tile as tile
from concourse import bacc

F32 = mybir.dt.float32
BF16 = mybir.dt.bfloat16
I16 = mybir.dt.int16
I32 = mybir.dt.int32
AF = mybir.ActivationFunctionType
ALU = mybir.AluOpType

# Problem constants (hardcoded per contract)
N = 100_000
E = 1_600_000
IN, HID, OUT, H = 256, 128, 64, 1
SLOPE = 0.2
NCORES = 8
P = 128
EPS = 1e-30
ECLAMP = 30.0       # logit clamp: keeps exp() finite on stale pad slots
NCHUNK = 4          # source-table chunks (rows per chunk must fit int16)


class Cfg:
    """Geometry, parameterized so small test instances can be built."""

    def __init__(self, n=N, e=E, fin=IN, hid=HID, out=OUT, ncores=NCORES):
        self.n, self.e, self.fin, self.hid, self.out = n, e, fin, hid, out
        self.ncores = ncores
        assert n % ncores == 0
        self.nl = n // ncores                  # dst nodes per core
        self.nbins = math.ceil(self.nl / P)    # bins per core
        self.nlp = self.nbins * P              # padded local nodes
        self.ntab = self.nlp * ncores          # rows in gathered tables
        self.nchunk = min(NCHUNK, ncores)
        assert self.ntab % self.nchunk == 0
        self.chrows = self.ntab // self.nchunk  # rows per source chunk
        assert self.chrows < 32768, "chunk rows must fit int16"
        assert fin % P == 0
        self.kt = fin // P                     # K-tiles for dense1


# ----------------------------------------------------------------------------
# Host-side graph preprocessing
# ----------------------------------------------------------------------------

def prep_graph(cfg: Cfg, edge_index: np.ndarray):
    """Bin-pack dsts, group edges by (bin, src chunk), build index arrays."""
    n, ncores, nl, nbins, nlp = cfg.n, cfg.ncores, cfg.nl, cfg.nbins, cfg.nlp
    nck, chrows = cfg.nchunk, cfg.chrows
    src = np.concatenate([edge_index[0], np.arange(n, dtype=np.int64)])
    dst = np.concatenate([edge_index[1], np.arange(n, dtype=np.int64)])

    # --- per-core bin-packing of dst nodes ---------------------------------
    slot_global = np.full(n, -1, dtype=np.int64)  # node -> row in table space
    deg_all = np.bincount(dst, minlength=n)
    import heapq
    for c in range(ncores):
        lo, hi = c * nl, (c + 1) * nl
        deg = deg_all[lo:hi]
        order = np.argsort(-deg, kind="stable")
        heap = [(0, 0, b) for b in range(nbins)]
        heapq.heapify(heap)
        stash = []
        for node in order:
            d = int(deg[node])
            while True:
                s, cnt, b = heapq.heappop(heap)
                if cnt < P:
                    break
                stash.append((s, cnt, b))
            slot_global[lo + node] = c * nlp + b * P + cnt
            heapq.heappush(heap, (s + d, cnt + 1, b))
            for it in stash:
                heapq.heappush(heap, it)
            stash.clear()

    # --- group edges by (core, bin, chunk) ---------------------------------
    sslot = slot_global[src]
    dslot = slot_global[dst]
    chunk = sslot // chrows
    binid = dslot // P                    # global bin id = core*nbins + bin
    key = binid * nck + chunk
    order = np.argsort(key, kind="stable")
    s_o, d_o, k_o = sslot[order], dslot[order], key[order]
    nkeys = ncores * nbins * nck
    cnts = np.bincount(k_o, minlength=nkeys).reshape(ncores, nbins, nck)
    offs = np.concatenate([[0], np.cumsum(cnts.reshape(-1))])

    # uniform-across-cores tiles per (bin, chunk)
    tbo = np.maximum(np.ceil(cnts / P).astype(np.int64).max(axis=0),
                     (cnts.max(axis=0) > 0))  # [nbins, nck]
    tbin = tbo.sum(axis=1)                 # [nbins] tiles per bin
    nslots = int(tbin.sum()) * P           # edge slots per core
    # uniform valid-index count per (bin, chunk): max across cores; slots
    # beyond it carry -1 indices, which the gather ucode skips (trailing
    # negatives emit no DMA descriptors).
    vcnt = np.maximum(cnts.max(axis=0), (tbo > 0)).astype(np.int64)

    # --- per-core index arrays ---------------------------------------------
    # gidx16: wrapped-16 int16 chunk-local src indices, [128, nslots//16]
    # dstcol: lane-major one-hot columns (f32),         [128, nslots//128]
    # onehotT: OT[d, slot] = 1 iff slot's dst row (bin-local) == d
    gidx16 = np.zeros((ncores, 128, nslots // 16), np.int16)
    dstcol = np.full((ncores, 128, nslots // 128), 200.0, np.float32)
    onehotT = np.zeros((ncores, 128, nslots), np.float32)
    for c in range(ncores):
        pos = 0  # slot position within the core's stream
        for b in range(nbins):
            rbase = b * P
            for o in range(nck):
                kk = int(cnts[c, b, o])
                so = offs[(c * nbins + b) * nck + o]
                slots = int(tbo[b, o]) * P
                if slots == 0:
                    continue
                j = np.arange(kk)
                jp = pos + j
                vc = int(vcnt[b, o])
                # [0:kk] real, [kk:vc] index-0 pads (uniform descriptor
                # count across cores), [vc:slots] -1 (skipped by ucode)
                g = np.full(slots, -1, np.int16)
                g[j] = (s_o[so:so + kk] - o * chrows).astype(np.int16)
                g[kk:vc] = 0
                jj = pos + np.arange(slots)
                gidx16[c, jj % 16, jj // 16] = g
                dloc = (d_o[so:so + kk] - (c * nlp + rbase)).astype(np.int64)
                dstcol[c, jp % 128, jp // 128] = dloc.astype(np.float32)
                onehotT[c, dloc, jp] = 1.0
                pos += slots
        assert pos == nslots
        # the Q7 gather ucode reads indices from its own 16-partition group:
        # replicate the wrapped-16 data across all 8 groups
        gidx16[c] = np.tile(gidx16[c, :16], (8, 1))

    # node permutation per core: slot s -> original node (or -1)
    perm = np.full((ncores, nlp), -1, dtype=np.int64)
    nodes = np.where(slot_global >= 0)[0]
    perm.reshape(-1)[slot_global[nodes]] = nodes

    return dict(
        tbo=tbo, tbin=tbin, vcnt=vcnt, nslots=nslots,
        slot_global=slot_global, perm=perm,
        gidx16=gidx16, dstcol=dstcol, onehotT=onehotT,
    )


# ----------------------------------------------------------------------------
# Device program builders (single SPMD program, data differs per core)
# ----------------------------------------------------------------------------

def _new_nc(cfg, nq=1):
    return bacc.Bacc("TRN2", target_bir_lowering=False, debug=False,
                     enable_asserts=False, num_devices=cfg.ncores,
                     num_swdge_queues=nq)


def build_dense1(cfg: Cfg, dt=F32):
    """xT [fin, nlp] -> XL1 [nlp, hid], XR1 [nlp, hid]."""
    nc = _new_nc(cfg)
    fin, hid, nlp, kt = cfg.fin, cfg.hid, cfg.nlp, cfg.kt
    xT = nc.dram_tensor("xT", [fin, nlp], F32, kind="ExternalInput")
    wl = nc.dram_tensor("wl", [fin, hid], F32, kind="ExternalInput")
    wr = nc.dram_tensor("wr", [fin, hid], F32, kind="ExternalInput")
    blB = nc.dram_tensor("blB", [P, hid], F32, kind="ExternalInput")
    brB = nc.dram_tensor("brB", [P, hid], F32, kind="ExternalInput")
    XL = nc.dram_tensor("XL1", [nlp, hid], dt, kind="ExternalOutput")
    XR = nc.dram_tensor("XR1", [nlp, hid], dt, kind="ExternalOutput")

    mtiles = nlp // P
    with tile.TileContext(nc) as tc:
        with tc.tile_pool(name="const", bufs=1) as cp, \
             tc.tile_pool(name="work", bufs=4) as wp, \
             tc.tile_pool(name="psum", bufs=4, space="PSUM") as pp:
            xk = cp.tile([P, kt, nlp], F32)
            nc.sync.dma_start(xk[:], xT[:].rearrange("(k p) n -> p k n", p=P))
            wl_sb = cp.tile([P, kt, hid], F32)
            nc.sync.dma_start(wl_sb[:], wl[:].rearrange("(k p) h -> p k h", p=P))
            wr_sb = cp.tile([P, kt, hid], F32)
            nc.sync.dma_start(wr_sb[:], wr[:].rearrange("(k p) h -> p k h", p=P))
            blB_sb = cp.tile([P, hid], F32)
            nc.sync.dma_start(blB_sb[:], blB[:])
            brB_sb = cp.tile([P, hid], F32)
            nc.sync.dma_start(brB_sb[:], brB[:])

            for m in range(mtiles):
                ms = slice(m * P, (m + 1) * P)
                psl = pp.tile([P, hid], F32, tag="psl")
                psr = pp.tile([P, hid], F32, tag="psr")
                for k in range(kt):
                    nc.tensor.matmul(psl[:], lhsT=xk[:, k, ms], rhs=wl_sb[:, k, :],
                                     start=(k == 0), stop=(k == kt - 1))
                for k in range(kt):
                    nc.tensor.matmul(psr[:], lhsT=xk[:, k, ms], rhs=wr_sb[:, k, :],
                                     start=(k == 0), stop=(k == kt - 1))
                ol = wp.tile([P, hid], dt, tag="ol")
                nc.vector.tensor_tensor(out=ol[:], in0=psl[:], in1=blB_sb[:], op=ALU.add)
                orr = wp.tile([P, hid], dt, tag="orr")
                nc.vector.tensor_tensor(out=orr[:], in0=psr[:], in1=brB_sb[:], op=ALU.add)
                nc.sync.dma_start(XL[ms, :], ol[:])
                nc.sync.dma_start(XR[ms, :], orr[:])
    nc.compile()
    return nc


def _edge_phase(nc, tc, cfg, pr, pools, tabs, consts, n_lay, kpos, finalize, dt):
    """Shared edge pipeline over bins (att folded into the tables).

    Gathered rows are 128 wide: n_lay layer blocks of feat=128//n_lay cols;
    within each block, columns with positive att come first (kpos[l] of
    them), as permuted host-side.  xr[dst] is reconstructed per tile on the
    TensorEngine from the SBUF-resident XR table via the host-baked one-hot
    OT.  finalize(b, psums): consume accumulated PSUM tiles per bin
    (cols [0:feat] numerator, col [feat] denominator).
    """
    from concourse import library_config
    nc.gpsimd.load_library(library_config.mlp)
    cp, gp, wp, pp = pools
    XLchunks, XR, GIDX, DCOL, OHT = tabs
    iotaRep_sb = consts
    tbo, tbin, vcnt = pr["tbo"], pr["tbin"], pr["vcnt"]
    nslots = pr["nslots"]
    nck, chrows = cfg.nchunk, cfg.chrows
    feat = P // n_lay
    Tmax = int(tbin.max())
    NQ = nc.num_swdge_queues
    qn = 0

    # whole-launch index array resident in SBUF
    gix = cp.tile([P, nslots // 16], I16)
    nc.sync.dma_start(gix[:], GIDX[:])
    dcl = cp.tile([P, nslots // 128, 1], dt)
    nc.sync.dma_start(dcl[:], DCOL[:])
    # local XR table resident: [p, bin, hid] with p = row within bin
    xrt = cp.tile([P, cfg.nbins, P], dt)
    nc.sync.dma_start(xrt[:], XR[:].rearrange("(b p) h -> p b h", p=P))

    bpos = np.concatenate([[0], np.cumsum(tbin)]) * P
    st = {}

    def stage1(b):
        """Gathers + OT stream + xr reconstruction (PE matmul, ACT copy)."""
        nonlocal qn
        pos = int(bpos[b])
        Tb = int(tbin[b])
        G = gp.tile([P, Tmax, P], dt, tag="G", name="G")
        OT = gp.tile([P, Tmax, P], dt, tag="OT", name="OT")
        Rs = gp.tile([P, Tmax, P], dt, tag="Rs", name="Rs")
        if b < 2:
            # first use of each pool buffer: clear so skipped (-1) pad
            # slots hold finite values, not uninitialized SBUF bits
            nc.vector.memset(G[:], 0.0)
        to = 0
        for o in range(nck):
            tt = int(tbo[b, o])
            if tt == 0:
                continue
            nidx = tt * P
            vc = int(vcnt[b, o])
            col = (pos + to * P) // 16
            nc.gpsimd.dma_gather(
                out_ap=G[:, to:to + tt, :],
                in_ap=XLchunks[o][:],
                idxs_ap=gix[:, col:col + nidx // 16],
                num_idxs=nidx, num_idxs_reg=vc, elem_size=P,
                single_packet=(nidx <= 1024), queue_num=qn % NQ)
            qn += 1
            to += tt
        assert to == Tb
        nc.sync.dma_start(OT[:, 0:Tb, :],
                          OHT[:, pos:pos + Tb * P].rearrange(
                              "d (t e) -> d t e", e=P))
        # xr[dst] per edge: R_t = OT_t^T @ XR_bin on the TensorEngine.
        # 4 tiles share one PSUM bank (slice 0's start=True zeroes the whole
        # bank; later slices accumulate onto zeroed regions), so one ACT
        # copy moves 4 tiles to SBUF (bf16) and the z-add runs 2x batched.
        for t0 in range(0, Tb, 4):
            gsz = min(4, Tb - t0)
            pR = pp.tile([P, 4, P], F32, tag="pR", name="pR")
            for i in range(gsz):
                nc.tensor.matmul(pR[:, i, :], lhsT=OT[:, t0 + i, :],
                                 rhs=xrt[:, b, :],
                                 start=(i == 0), stop=(i == gsz - 1))
            nc.scalar.activation(out=Rs[:, t0:t0 + gsz, :],
                                 in_=pR[:, 0:gsz, :], func=AF.Copy)
        st[b] = dict(G=G, OT=OT, Rs=Rs, pos=pos, Tb=Tb)

    def stage2(b):
        """Logits + softmax weights (DVE + ACT)."""
        s = st[b]
        G, Rs, pos, Tb = s["G"], s["Rs"], s["pos"], s["Tb"]
        dcol = dcl[:, pos // P:(pos + Tb * P) // P, :]   # [P, Tb, 1]
        # z = xl'[src] + xr'[dst]  (att-scaled space)
        Z = wp.tile([P, Tmax, P], dt, tag="Z", name="Z")
        nc.vector.tensor_tensor(out=Z[:, 0:Tb, :], in0=G[:, 0:Tb, :],
                                in1=Rs[:, 0:Tb, :], op=ALU.add)
        # signed-att leaky-relu: positive-att cols get lrelu(z) = max(z,.2z),
        # negative-att cols get min(z,.2z) = Prelu(alpha=1/s)(s*z), s=.2 --
        # then the plain row-sum per layer IS the logit.
        U = wp.tile([P, Tmax, P], dt, tag="U", name="U")
        for l in range(n_lay):
            k = kpos[l]
            c0 = l * feat
            if k > 0:
                nc.scalar.activation(out=U[:, 0:Tb, c0:c0 + k],
                                     in_=Z[:, 0:Tb, c0:c0 + k],
                                     func=AF.Prelu, alpha=SLOPE)
            if k < feat:
                nc.scalar.activation(out=U[:, 0:Tb, c0 + k:c0 + feat],
                                     in_=Z[:, 0:Tb, c0 + k:c0 + feat],
                                     func=AF.Prelu, alpha=1.0 / SLOPE,
                                     scale=SLOPE)
        e = wp.tile([P, Tmax, n_lay, 1], F32, tag="e", name="e")
        nc.vector.tensor_reduce(
            out=e[:, 0:Tb, :, :],
            in_=U[:, 0:Tb, :].rearrange("p t (l f) -> p t l f", l=n_lay),
            axis=mybir.AxisListType.X, op=ALU.add)
        # clamp: skipped pad slots carry stale-but-finite data whose logit
        # can be large; exp must stay finite so 0*exp(e) == 0 in the matmul
        nc.vector.tensor_scalar(out=e[:, 0:Tb, :, :], in0=e[:, 0:Tb, :, :],
                                scalar1=ECLAMP, scalar2=None, op0=ALU.min)
        ee = wp.tile([P, Tmax, n_lay, 1], dt, tag="ee", name="ee")
        nc.scalar.activation(out=ee[:, 0:Tb, :, :], in_=e[:, 0:Tb, :, :],
                             func=AF.Exp)

        # one-hot (dst-col) and softmax-weighted source rows
        O = wp.tile([P, Tmax, 1, P], dt, tag="O", name="O")
        nc.vector.tensor_tensor(
            out=O[:, 0:Tb, 0, :],
            in0=dcol.to_broadcast([P, Tb, P]),
            in1=iotaRep_sb[:, 0:Tb, :], op=ALU.is_equal)
        # W_l = ee_l * G_l: per-layer halves, so the weighting pass covers
        # 128 cols total regardless of n_lay (the one-hot O is shared)
        W = wp.tile([P, Tmax, P], dt, tag="W", name="W")
        Wv = W[:, 0:Tb, :].rearrange("p t (l f) -> p t l f", l=n_lay)
        nc.vector.tensor_tensor(
            out=Wv[:],
            in0=G[:, 0:Tb, :].rearrange("p t (l f) -> p t l f", l=n_lay),
            in1=ee[:, 0:Tb, :, :].to_broadcast([P, Tb, n_lay, feat]),
            op=ALU.mult)
        s.update(O=O, W=W, ee=ee)

    def stage3(b):
        """Accumulate numerator/denominator (PE) and finalize the bin."""
        s = st.pop(b)
        O, W, ee, Tb = s["O"], s["W"], s["ee"], s["Tb"]
        # numerator cols [0:feat], denominator col [feat]: one PSUM bank per
        # layer.  num's start=True zeroes the whole bank, so the den matmul
        # always runs start=False and lands on a zeroed column.  All matmuls
        # of a tile share lhsT = the one-hot O_t.
        pn = [pp.tile([P, feat + n_lay if l == 0 else feat], F32,
                      tag=f"pn{l}", name=f"pn{l}")
              for l in range(n_lay)]
        for t in range(Tb):
            for l in range(n_lay):
                nc.tensor.matmul(
                    pn[l][:, 0:feat], lhsT=O[:, t, 0, :],
                    rhs=W[:, t, l * feat:(l + 1) * feat],
                    start=(t == 0), stop=(t == Tb - 1))
            nc.tensor.matmul(
                pn[0][:, feat:feat + n_lay], lhsT=O[:, t, 0, :],
                rhs=ee[:, t, :, 0], start=False, stop=(t == Tb - 1))
        finalize(b, pn)

    # 1-bin software pipeline: keep the TensorEngine fed (its stream is
    # in-order, so bin b+1's xr matmuls must be emitted before bin b's
    # accumulation matmuls to overlap with the DVE/ACT middle stages).
    stage1(0)
    for b in range(cfg.nbins):
        if b + 1 < cfg.nbins:
            stage1(b + 1)
        stage2(b)
        stage3(b)


def _load_consts(nc, cp, names_shapes):
    out = []
    for name, shape, dt in names_shapes:
        dram = nc.dram_tensor(name, shape, dt, kind="ExternalInput")
        sb = cp.tile(shape, dt, name=name + "_sb")
        nc.sync.dma_start(sb[:], dram[:])
        out.append(sb)
    return out


def build_edge1(cfg: Cfg, pr, kpos, dt=F32, nq=1):
    """Edge phase of layer 1 + dense transforms of layers 2/3."""
    nc = _new_nc(cfg, nq)
    hid, out, nlp, nbins = cfg.hid, cfg.out, cfg.nlp, cfg.nbins
    nslots = pr["nslots"]
    Tmax = int(pr["tbin"].max())
    XLchunks = [nc.dram_tensor(f"XL1c{o}", [cfg.chrows, hid], dt,
                               kind="ExternalInput") for o in range(cfg.nchunk)]
    XR = nc.dram_tensor("XR1", [nlp, hid], dt, kind="ExternalInput")
    GIDX = nc.dram_tensor("gidx", [P, nslots // 16], I16, kind="ExternalInput")
    DCOL = nc.dram_tensor("dcol", [P, nslots // 128, 1], dt, kind="ExternalInput")
    OHT = nc.dram_tensor("oht", [P, nslots], dt, kind="ExternalInput")
    XL23 = nc.dram_tensor("XL23", [nlp, P], dt, kind="ExternalOutput")
    XR23 = nc.dram_tensor("XR23", [nlp, P], dt, kind="ExternalOutput")

    with tile.TileContext(nc) as tc:
        with tc.tile_pool(name="const", bufs=1) as cp, \
             tc.tile_pool(name="gath", bufs=2) as gp, \
             tc.tile_pool(name="work", bufs=2) as wp, \
             tc.tile_pool(name="psum", bufs=2, space="PSUM") as pp, \
             tc.tile_pool(name="psfin", bufs=2, space="PSUM") as pf:
            (iotaRep_sb, winv1B_sb, b1B_sb, ident_sb, w23lr_sb,
             b23lr_sb) = _load_consts(nc, cp, [
                 ("iotaRep", [P, Tmax, P], dt),
                 ("winv1B", [P, hid], F32),
                 ("b1B", [P, hid], F32),
                 ("identB", [P, P], F32),
                 ("w23lr", [hid, 2 * P], F32),
                 ("b23lrB", [P, 2 * P], F32)])

            def finalize(b, pn):
                ms = slice(b * P, (b + 1) * P)
                d = wp.tile([P, 1], F32, tag="d", name="d")
                nc.vector.tensor_scalar_add(d[:], pn[0][:, hid:hid + 1], EPS)
                r = wp.tile([P, 1], F32, tag="r", name="r")
                nc.vector.reciprocal(r[:], d[:])
                h = wp.tile([P, hid], F32, tag="h", name="h")
                # h = (num * 1/den) * winv  (unscale att, permuted order)
                nc.vector.scalar_tensor_tensor(
                    out=h[:], in0=pn[0][:, 0:hid], scalar=r[:],
                    in1=winv1B_sb[:], op0=ALU.mult, op1=ALU.mult)
                nc.vector.tensor_tensor(out=h[:], in0=h[:], in1=b1B_sb[:], op=ALU.add)
                nc.scalar.activation(out=h[:], in_=h[:], func=AF.Relu)
                pst = pf.tile([P, P], F32, tag="pst", name="pst")
                nc.tensor.transpose(out=pst[:], in_=h[:], identity=ident_sb[:])
                hT = wp.tile([P, P], F32, tag="hT", name="hT")
                nc.scalar.activation(out=hT[:], in_=pst[:], func=AF.Copy)
                psB = pf.tile([P, 2 * P], F32, tag="psB", name="psB")
                nc.tensor.matmul(psB[:], lhsT=hT[:, 0:hid], rhs=w23lr_sb[:],
                                 start=True, stop=True)
                olr = wp.tile([P, 2 * P], dt, tag="olr", name="olr")
                nc.vector.tensor_tensor(out=olr[:], in0=psB[:], in1=b23lr_sb[:],
                                        op=ALU.add)
                nc.sync.dma_start(XL23[ms, :], olr[:, 0:P])
                nc.sync.dma_start(XR23[ms, :], olr[:, P:2 * P])

            _edge_phase(nc, tc, cfg, pr, (cp, gp, wp, pp),
                        (XLchunks, XR, GIDX, DCOL, OHT),
                        iotaRep_sb, 1, kpos, finalize, dt)
    nc.compile()
    return nc


def build_edge23(cfg: Cfg, pr, kpos, dt=F32, nq=1):
    """Edge phases of layers 2 and 3 (shared gather)."""
    nc = _new_nc(cfg, nq)
    out, nlp, nbins = cfg.out, cfg.nlp, cfg.nbins
    nslots = pr["nslots"]
    Tmax = int(pr["tbin"].max())
    XLchunks = [nc.dram_tensor(f"XL23c{o}", [cfg.chrows, P], dt,
                               kind="ExternalInput") for o in range(cfg.nchunk)]
    XR = nc.dram_tensor("XR23", [nlp, P], dt, kind="ExternalInput")
    GIDX = nc.dram_tensor("gidx", [P, nslots // 16], I16, kind="ExternalInput")
    DCOL = nc.dram_tensor("dcol", [P, nslots // 128, 1], dt, kind="ExternalInput")
    OHT = nc.dram_tensor("oht", [P, nslots], dt, kind="ExternalInput")
    MU = nc.dram_tensor("MU", [nlp, out], F32, kind="ExternalOutput")
    LV = nc.dram_tensor("LV", [nlp, out], F32, kind="ExternalOutput")

    with tile.TileContext(nc) as tc:
        with tc.tile_pool(name="const", bufs=1) as cp, \
             tc.tile_pool(name="gath", bufs=2) as gp, \
             tc.tile_pool(name="work", bufs=2) as wp, \
             tc.tile_pool(name="psum", bufs=2, space="PSUM") as pp:
            (iotaRep_sb, winvmuB_sb, winvlvB_sb, bmu_sb, blv_sb) = _load_consts(
                nc, cp, [
                    ("iotaRep", [P, Tmax, P], dt),
                    ("winvmuB", [P, out], F32),
                    ("winvlvB", [P, out], F32),
                    ("bmuB", [P, out], F32),
                    ("blvB", [P, out], F32)])

            def finalize(b, pn):
                ms = slice(b * P, (b + 1) * P)
                for l, (winv, bias, dest, tg) in enumerate(
                        ((winvmuB_sb, bmu_sb, MU, "mu"),
                         (winvlvB_sb, blv_sb, LV, "lv"))):
                    d = wp.tile([P, 1], F32, tag=f"d{tg}", name="d")
                    nc.vector.tensor_scalar_add(d[:], pn[0][:, out + l:out + l + 1],
                                                EPS)
                    r = wp.tile([P, 1], F32, tag=f"r{tg}", name="r")
                    nc.vector.reciprocal(r[:], d[:])
                    o = wp.tile([P, out], F32, tag=f"o{tg}", name="o")
                    nc.vector.scalar_tensor_tensor(
                        out=o[:], in0=pn[l][:, 0:out], scalar=r[:],
                        in1=winv[:], op0=ALU.mult, op1=ALU.mult)
                    nc.vector.tensor_tensor(out=o[:], in0=o[:], in1=bias[:], op=ALU.add)
                    nc.sync.dma_start(dest[ms, :], o[:])

            _edge_phase(nc, tc, cfg, pr, (cp, gp, wp, pp),
                        (XLchunks, XR, GIDX, DCOL, OHT),
                        iotaRep_sb, 2, kpos, finalize, dt)
    nc.compile()
    return nc


# ----------------------------------------------------------------------------
# Host orchestration
# ----------------------------------------------------------------------------

def _bb(v, rows=P):
    """Broadcast a 1-D row vector to [rows, len] f32."""
    v = np.asarray(v, np.float32).reshape(1, -1)
    return np.ascontiguousarray(np.broadcast_to(v, (rows, v.shape[1])))


def _hw_runner(nc, in_maps, cfg, trace=False):
    from concourse import bass_utils
    r = bass_utils.run_bass_kernel_spmd(
        nc, in_maps, core_ids=list(range(cfg.ncores)), trace=trace)
    return r.results, r.exec_time_ns


class _State:
    """Cached compiled programs + prep, keyed by edge structure."""
    key = None
    progs = None
    prep = None
    fold = None


EDT = BF16 if not int(os.environ.get("GAT_F32", "0")) else F32
NQUEUES = 4


def fold_weights(w):
    """Fold att into the tables: scale columns by att (signed), permute so
    positive-att columns come first in each layer block.  Returns folded
    weights + the inverse data needed at finalize / host postprocess."""
    f = {}
    perms, kpos = {}, {}
    for name, fo in (("sh", HID), ("mu", OUT), ("lv", OUT)):
        a = np.asarray(w[f"{name}_att"], np.float32).reshape(-1)  # [fo]
        pi = np.argsort(a <= 0, kind="stable")  # positive cols first
        perms[name] = pi
        kpos[name] = int((a > 0).sum())
        f[f"{name}_a"] = np.ascontiguousarray(a[pi])  # signed att, permuted
        f[f"{name}_Wl"] = np.ascontiguousarray(
            (np.asarray(w[f"{name}_Wl"], np.float32) * a)[:, pi])
        f[f"{name}_Wr"] = np.ascontiguousarray(
            (np.asarray(w[f"{name}_Wr"], np.float32) * a)[:, pi])
        f[f"{name}_bl"] = (np.asarray(w[f"{name}_bl"], np.float32) * a)[pi]
        f[f"{name}_br"] = (np.asarray(w[f"{name}_br"], np.float32) * a)[pi]
        f[f"{name}_winv"] = 1.0 / f[f"{name}_a"]
        f[f"{name}_b"] = np.asarray(w[f"{name}_b"], np.float32)[pi]
    # layer-1 permutation also permutes h's columns -> permute W23 rows
    pi1 = perms["sh"]
    f["mu_Wl"] = np.ascontiguousarray(f["mu_Wl"][pi1])
    f["mu_Wr"] = np.ascontiguousarray(f["mu_Wr"][pi1])
    f["lv_Wl"] = np.ascontiguousarray(f["lv_Wl"][pi1])
    f["lv_Wr"] = np.ascontiguousarray(f["lv_Wr"][pi1])
    f["perms"] = perms
    f["kpos"] = kpos
    return f


def build_progs(cfg, pr, fold, dt=None, nq=None):
    dt = EDT if dt is None else dt
    nq = NQUEUES if nq is None else nq
    kp = fold["kpos"]
    return dict(
        dense1=build_dense1(cfg, dt),
        edge1=build_edge1(cfg, pr, [kp["sh"]], dt, nq),
        edge23=build_edge23(cfg, pr, [kp["mu"], kp["lv"]], dt, nq),
    )


def forward(cfg, x, ei_unused, w, f, pr, progs, runner, dt=None):
    dt = EDT if dt is None else dt
    ndt = mybir.dt.np(dt)
    perm = pr["perm"]                    # [ncores, nlp] node ids or -1
    Tmax = int(pr["tbin"].max())
    profile = {}
    dcol_in = [np.ascontiguousarray(pr["dstcol"][c][:, :, None].astype(ndt))
               for c in range(cfg.ncores)]
    oht_in = [np.ascontiguousarray(pr["onehotT"][c].astype(ndt))
              for c in range(cfg.ncores)]

    hid, out, nlp, ntab = cfg.hid, cfg.out, cfg.nlp, cfg.ntab

    # ---- launch A: dense1 (att-scaled, sign-permuted weights) -------------
    in_maps = []
    for c in range(cfg.ncores):
        xs = np.zeros((nlp, cfg.fin), np.float32)
        sel = perm[c] >= 0
        xs[sel] = x[perm[c][sel]]
        in_maps.append(dict(
            xT=np.ascontiguousarray(xs.T), wl=f["sh_Wl"], wr=f["sh_Wr"],
            blB=_bb(f["sh_bl"]), brB=_bb(f["sh_br"])))
    rA, profile["A"] = runner(progs["dense1"], in_maps, cfg)
    XL1full = np.concatenate([rA[c]["XL1"] for c in range(cfg.ncores)])
    XL1ch = {f"XL1c{o}": np.ascontiguousarray(
        XL1full[o * cfg.chrows:(o + 1) * cfg.chrows])
        for o in range(cfg.nchunk)}
    XR1 = [rA[c]["XR1"] for c in range(cfg.ncores)]

    # ---- launch B: edge1 + dense23 ----------------------------------------
    iotaRep = np.ascontiguousarray(np.broadcast_to(
        np.arange(P, dtype=np.float32), (P, Tmax, P))).astype(ndt)
    w23l = np.concatenate([f["mu_Wl"], f["lv_Wl"]], axis=1)
    w23r = np.concatenate([f["mu_Wr"], f["lv_Wr"]], axis=1)
    b23l = np.concatenate([f["mu_bl"], f["lv_bl"]])
    b23r = np.concatenate([f["mu_br"], f["lv_br"]])
    ident = np.eye(P, dtype=np.float32)
    in_maps = []
    for c in range(cfg.ncores):
        in_maps.append(dict(
            XR1=XR1[c], **XL1ch,
            gidx=pr["gidx16"][c], dcol=dcol_in[c], oht=oht_in[c],
            iotaRep=iotaRep, winv1B=_bb(f["sh_winv"]), b1B=_bb(f["sh_b"]),
            identB=ident,
            w23lr=np.ascontiguousarray(np.concatenate([w23l, w23r], axis=1)),
            b23lrB=_bb(np.concatenate([b23l, b23r]))))
    rB, profile["B"] = runner(progs["edge1"], in_maps, cfg)
    XL23full = np.concatenate([rB[c]["XL23"] for c in range(cfg.ncores)])
    XL23ch = {f"XL23c{o}": np.ascontiguousarray(
        XL23full[o * cfg.chrows:(o + 1) * cfg.chrows])
        for o in range(cfg.nchunk)}
    XR23 = [rB[c]["XR23"] for c in range(cfg.ncores)]

    # ---- launch C: edge23 --------------------------------------------------
    in_maps = []
    for c in range(cfg.ncores):
        in_maps.append(dict(
            XR23=XR23[c], **XL23ch,
            gidx=pr["gidx16"][c], dcol=dcol_in[c], oht=oht_in[c],
            iotaRep=iotaRep,
            winvmuB=_bb(f["mu_winv"]), winvlvB=_bb(f["lv_winv"]),
            bmuB=_bb(f["mu_b"]), blvB=_bb(f["lv_b"])))
    rC, profile["C"] = runner(progs["edge23"], in_maps, cfg)

    MU = np.concatenate([rC[c]["MU"] for c in range(cfg.ncores)])
    LV = np.concatenate([rC[c]["LV"] for c in range(cfg.ncores)])
    # undo the sign permutation of the output columns
    mu_u = np.empty_like(MU)
    mu_u[:, f["perms"]["mu"]] = MU
    lv_u = np.empty_like(LV)
    lv_u[:, f["perms"]["lv"]] = LV
    mu = mu_u[pr["slot_global"]]
    lv = lv_u[pr["slot_global"]]
    return (mu, lv), profile


def kernel(**inputs):
    cfg = Cfg()
    x = np.asarray(inputs["x"], np.float32)
    ei = np.asarray(inputs["edge_index"]).astype(np.int64)
    w = {k: np.asarray(v, np.float32) for k, v in inputs.items()
         if k not in ("x", "edge_index")}

    fold = fold_weights(w)
    key = (hash(ei.tobytes()), tuple(sorted(fold["kpos"].items())))
    if _State.key != key:
        pr = prep_graph(cfg, ei)
        _State.prep = pr
        _State.progs = build_progs(cfg, pr, fold)
        _State.key = key
    _State.fold = fold

    trace = bool(int(os.environ.get("GAT_TRACE", "0")))
    runner = functools.partial(_hw_runner, trace=trace)
    (mu, lv), profile = forward(cfg, x, ei, w, _State.fold, _State.prep,
                                _State.progs, runner)
    kernel._last_profile = profile
    return (mu, lv)


kernel._last_profile = None


# revision 21
# speedup vs baseline: 1.5539x; 1.4329x over previous
# GATv2 encoder (3x GATv2Conv, H=1) on 8 Trainium2 NeuronCores.
#
# Sharding: nodes partitioned by dst across 8 cores (graph parallel).
# Edge work per core is organized as 98 "bins" of <=128 dst nodes each
# (host-side bin-packing balances edge counts); a bin's edges are grouped
# by source-table chunk (4 chunks of 25088 rows so indices fit int16) and
# padded to 128-edge tiles.  Per bin: batched dma_gather of att-scaled
# source rows xl'[src] (trailing -1 indices skip pad descriptors), xr[dst]
# reconstructed on the TensorEngine from the SBUF-resident local XR table
# via a host-baked one-hot (OT), leaky-relu logits on the ScalarEngine
# (att folded into the tables; columns permuted by sign(att) host-side),
# and per-tile one-hot matmuls accumulating softmax numerator + denominator
# in one PSUM bank.  Host gathers per-core dense outputs between launches.
import os
import sys
import math
import functools
import numpy as np

for _p in ("/opt/trn_rl_repo",):
    if _p not in sys.path and os.path.isdir(_p):
        sys.path.insert(0, _p)

import concourse.bass as bass
import concourse.mybir as mybir
import concourse.tile as tile
from concourse import bacc

F32 = mybir.dt.float32
BF16 = mybir.dt.bfloat16
I16 = mybir.dt.int16
I32 = mybir.dt.int32
AF = mybir.ActivationFunctionType
ALU = mybir.AluOpType

# Problem constants (hardcoded per contract)
N = 100_000
E = 1_600_000
IN, HID, OUT, H = 256, 128, 64, 1
SLOPE = 0.2
NCORES = 8
P = 128
EPS = 1e-30
ECLAMP = 30.0       # logit clamp: keeps exp() finite on stale pad slots
NCHUNK = 4          # source-table chunks (rows per chunk must fit int16)


class Cfg:
    """Geometry, parameterized so small test instances can be built."""

    def __init__(self, n=N, e=E, fin=IN, hid=HID, out=OUT, ncores=NCORES):
        self.n, self.e, self.fin, self.hid, self.out = n, e, fin, hid, out
        self.ncores = ncores
        assert n % ncores == 0
        self.nl = n // ncores                  # dst nodes per core
        self.nbins = math.ceil(self.nl / P)    # bins per core
        self.nlp = self.nbins * P              # padded local nodes
        self.ntab = self.nlp * ncores          # rows in gathered tables
        self.nchunk = min(NCHUNK, ncores)
        assert self.ntab % self.nchunk == 0
        self.chrows = self.ntab // self.nchunk  # rows per source chunk
        assert self.chrows < 32768, "chunk rows must fit int16"
        assert fin % P == 0
        self.kt = fin // P                     # K-tiles for dense1


# ----------------------------------------------------------------------------
# Host-side graph preprocessing
# ----------------------------------------------------------------------------

def prep_graph(cfg: Cfg, edge_index: np.ndarray):
    """Bin-pack dsts, group edges by (bin, src chunk), build index arrays."""
    n, ncores, nl, nbins, nlp = cfg.n, cfg.ncores, cfg.nl, cfg.nbins, cfg.nlp
    nck, chrows = cfg.nchunk, cfg.chrows
    src = np.concatenate([edge_index[0], np.arange(n, dtype=np.int64)])
    dst = np.concatenate([edge_index[1], np.arange(n, dtype=np.int64)])

    # --- per-core bin-packing of dst nodes ---------------------------------
    slot_global = np.full(n, -1, dtype=np.int64)  # node -> row in table space
    deg_all = np.bincount(dst, minlength=n)
    import heapq
    for c in range(ncores):
        lo, hi = c * nl, (c + 1) * nl
        deg = deg_all[lo:hi]
        order = np.argsort(-deg, kind="stable")
        heap = [(0, 0, b) for b in range(nbins)]
        heapq.heapify(heap)
        stash = []
        for node in order:
            d = int(deg[node])
            while True:
                s, cnt, b = heapq.heappop(heap)
                if cnt < P:
                    break
                stash.append((s, cnt, b))
            slot_global[lo + node] = c * nlp + b * P + cnt
            heapq.heappush(heap, (s + d, cnt + 1, b))
            for it in stash:
                heapq.heappush(heap, it)
            stash.clear()

    # --- group edges by (core, bin, chunk) ---------------------------------
    sslot = slot_global[src]
    dslot = slot_global[dst]
    chunk = sslot // chrows
    binid = dslot // P                    # global bin id = core*nbins + bin
    key = binid * nck + chunk
    order = np.argsort(key, kind="stable")
    s_o, d_o, k_o = sslot[order], dslot[order], key[order]
    nkeys = ncores * nbins * nck
    cnts = np.bincount(k_o, minlength=nkeys).reshape(ncores, nbins, nck)
    offs = np.concatenate([[0], np.cumsum(cnts.reshape(-1))])

    # uniform-across-cores tiles per (bin, chunk)
    tbo = np.maximum(np.ceil(cnts / P).astype(np.int64).max(axis=0),
                     (cnts.max(axis=0) > 0))  # [nbins, nck]
    tbin = tbo.sum(axis=1)                 # [nbins] tiles per bin
    nslots = int(tbin.sum()) * P           # edge slots per core
    # uniform valid-index count per (bin, chunk): max across cores; slots
    # beyond it carry -1 indices, which the gather ucode skips (trailing
    # negatives emit no DMA descriptors).
    vcnt = np.maximum(cnts.max(axis=0), (tbo > 0)).astype(np.int64)

    # --- per-core index arrays ---------------------------------------------
    # gidx16: wrapped-16 int16 chunk-local src indices, [128, nslots//16]
    # dstcol: lane-major one-hot columns (f32),         [128, nslots//128]
    # onehotT: OT[d, slot] = 1 iff slot's dst row (bin-local) == d
    gidx16 = np.zeros((ncores, 128, nslots // 16), np.int16)
    dstcol = np.full((ncores, 128, nslots // 128), 200.0, np.float32)
    onehotT = np.zeros((ncores, 128, nslots), np.float32)
    for c in range(ncores):
        pos = 0  # slot position within the core's stream
        for b in range(nbins):
            rbase = b * P
            for o in range(nck):
                kk = int(cnts[c, b, o])
                so = offs[(c * nbins + b) * nck + o]
                slots = int(tbo[b, o]) * P
                if slots == 0:
                    continue
                j = np.arange(kk)
                jp = pos + j
                vc = int(vcnt[b, o])
                # [0:kk] real, [kk:vc] index-0 pads (uniform descriptor
                # count across cores), [vc:slots] -1 (skipped by ucode)
                g = np.full(slots, -1, np.int16)
                g[j] = (s_o[so:so + kk] - o * chrows).astype(np.int16)
                g[kk:vc] = 0
                jj = pos + np.arange(slots)
                gidx16[c, jj % 16, jj // 16] = g
                dloc = (d_o[so:so + kk] - (c * nlp + rbase)).astype(np.int64)
                dstcol[c, jp % 128, jp // 128] = dloc.astype(np.float32)
                onehotT[c, dloc, jp] = 1.0
                pos += slots
        assert pos == nslots
        # the Q7 gather ucode reads indices from its own 16-partition group:
        # replicate the wrapped-16 data across all 8 groups
        gidx16[c] = np.tile(gidx16[c, :16], (8, 1))

    # node permutation per core: slot s -> original node (or -1)
    perm = np.full((ncores, nlp), -1, dtype=np.int64)
    nodes = np.where(slot_global >= 0)[0]
    perm.reshape(-1)[slot_global[nodes]] = nodes

    return dict(
        tbo=tbo, tbin=tbin, vcnt=vcnt, nslots=nslots,
        slot_global=slot_global, perm=perm,
        gidx16=gidx16, dstcol=dstcol, onehotT=onehotT,
    )


# ----------------------------------------------------------------------------
# Device program builders (single SPMD program, data differs per core)
# ----------------------------------------------------------------------------

def _new_nc(cfg, nq=1):
    return bacc.Bacc("TRN2", target_bir_lowering=False, debug=False,
                     enable_asserts=False, num_devices=cfg.ncores,
                     num_swdge_queues=nq)


def build_dense1(cfg: Cfg, dt=F32):
    """xT [fin, nlp] -> XL1 [nlp, hid], XR1 [nlp, hid]."""
    nc = _new_nc(cfg)
    fin, hid, nlp, kt = cfg.fin, cfg.hid, cfg.nlp, cfg.kt
    xT = nc.dram_tensor("xT", [fin, nlp], F32, kind="ExternalInput")
    wl = nc.dram_tensor("wl", [fin, hid], F32, kind="ExternalInput")
    wr = nc.dram_tensor("wr", [fin, hid], F32, kind="ExternalInput")
    blB = nc.dram_tensor("blB", [P, hid], F32, kind="ExternalInput")
    brB = nc.dram_tensor("brB", [P, hid], F32, kind="ExternalInput")
    XL = nc.dram_tensor("XL1", [nlp, hid], dt, kind="ExternalOutput")
    XR = nc.dram_tensor("XR1", [nlp, hid], dt, kind="ExternalOutput")

    mtiles = nlp // P
    with tile.TileContext(nc) as tc:
        with tc.tile_pool(name="const", bufs=1) as cp, \
             tc.tile_pool(name="work", bufs=4) as wp, \
             tc.tile_pool(name="psum", bufs=4, space="PSUM") as pp:
            xk = cp.tile([P, kt, nlp], F32)
            nc.sync.dma_start(xk[:], xT[:].rearrange("(k p) n -> p k n", p=P))
            wl_sb = cp.tile([P, kt, hid], F32)
            nc.sync.dma_start(wl_sb[:], wl[:].rearrange("(k p) h -> p k h", p=P))
            wr_sb = cp.tile([P, kt, hid], F32)
            nc.sync.dma_start(wr_sb[:], wr[:].rearrange("(k p) h -> p k h", p=P))
            blB_sb = cp.tile([P, hid], F32)
            nc.sync.dma_start(blB_sb[:], blB[:])
            brB_sb = cp.tile([P, hid], F32)
            nc.sync.dma_start(brB_sb[:], brB[:])

            for m in range(mtiles):
                ms = slice(m * P, (m + 1) * P)
                psl = pp.tile([P, hid], F32, tag="psl")
                psr = pp.tile([P, hid], F32, tag="psr")
                for k in range(kt):
                    nc.tensor.matmul(psl[:], lhsT=xk[:, k, ms], rhs=wl_sb[:, k, :],
                                     start=(k == 0), stop=(k == kt - 1))
                for k in range(kt):
                    nc.tensor.matmul(psr[:], lhsT=xk[:, k, ms], rhs=wr_sb[:, k, :],
                                     start=(k == 0), stop=(k == kt - 1))
                ol = wp.tile([P, hid], dt, tag="ol")
                nc.vector.tensor_tensor(out=ol[:], in0=psl[:], in1=blB_sb[:], op=ALU.add)
                orr = wp.tile([P, hid], dt, tag="orr")
                nc.vector.tensor_tensor(out=orr[:], in0=psr[:], in1=brB_sb[:], op=ALU.add)
                nc.sync.dma_start(XL[ms, :], ol[:])
                nc.sync.dma_start(XR[ms, :], orr[:])
    nc.compile()
    return nc


def _edge_phase(nc, tc, cfg, pr, pools, tabs, consts, n_lay, kpos, finalize, dt):
    """Shared edge pipeline over bins (att folded into the tables).

    Gathered rows are 128 wide: n_lay layer blocks of feat=128//n_lay cols;
    within each block, columns with positive att come first (kpos[l] of
    them), as permuted host-side.  xr[dst] is reconstructed per tile on the
    TensorEngine from the SBUF-resident XR table via the host-baked one-hot
    OT.  finalize(b, psums): consume accumulated PSUM tiles per bin
    (cols [0:feat] numerator, col [feat] denominator).
    """
    from concourse import library_config
    nc.gpsimd.load_library(library_config.mlp)
    cp, gp, wp, pp = pools
    XLchunks, XR, GIDX, DCOL, OHT = tabs
    iotaRep_sb = consts
    tbo, tbin, vcnt = pr["tbo"], pr["tbin"], pr["vcnt"]
    nslots = pr["nslots"]
    nck, chrows = cfg.nchunk, cfg.chrows
    feat = P // n_lay
    Tmax = int(tbin.max())
    NQ = nc.num_swdge_queues
    qn = 0

    # whole-launch index array resident in SBUF
    gix = cp.tile([P, nslots // 16], I16)
    nc.sync.dma_start(gix[:], GIDX[:])
    dcl = cp.tile([P, nslots // 128, 1], dt)
    nc.sync.dma_start(dcl[:], DCOL[:])
    # local XR table resident: [p, bin, hid] with p = row within bin
    xrt = cp.tile([P, cfg.nbins, P], dt)
    nc.sync.dma_start(xrt[:], XR[:].rearrange("(b p) h -> p b h", p=P))

    bpos = np.concatenate([[0], np.cumsum(tbin)]) * P
    st = {}

    def stage1(b):
        """Gathers + OT stream + xr reconstruction (PE matmul, ACT copy)."""
        nonlocal qn
        pos = int(bpos[b])
        Tb = int(tbin[b])
        G = gp.tile([P, Tmax, P], dt, tag="G", name="G")
        OT = gp.tile([P, Tmax, P], dt, tag="OT", name="OT")
        Rs = gp.tile([P, Tmax, P], dt, tag="Rs", name="Rs")
        if b < 2:
            # first use of each pool buffer: clear so skipped (-1) pad
            # slots hold finite values, not uninitialized SBUF bits
            nc.vector.memset(G[:], 0.0)
        to = 0
        for o in range(nck):
            tt = int(tbo[b, o])
            if tt == 0:
                continue
            nidx = tt * P
            vc = int(vcnt[b, o])
            col = (pos + to * P) // 16
            nc.gpsimd.dma_gather(
                out_ap=G[:, to:to + tt, :],
                in_ap=XLchunks[o][:],
                idxs_ap=gix[:, col:col + nidx // 16],
                num_idxs=nidx, num_idxs_reg=vc, elem_size=P,
                single_packet=(nidx <= 1024), queue_num=qn % NQ)
            qn += 1
            to += tt
        assert to == Tb
        nc.sync.dma_start(OT[:, 0:Tb, :],
                          OHT[:, pos:pos + Tb * P].rearrange(
                              "d (t e) -> d t e", e=P))
        # xr[dst] per edge: R_t = OT_t^T @ XR_bin on the TensorEngine.
        # 4 tiles share one PSUM bank (slice 0's start=True zeroes the whole
        # bank; later slices accumulate onto zeroed regions), so one ACT
        # copy moves 4 tiles to SBUF (bf16) and the z-add runs 2x batched.
        for t0 in range(0, Tb, 4):
            gsz = min(4, Tb - t0)
            pR = pp.tile([P, 4, P], F32, tag="pR", name="pR")
            for i in range(gsz):
                nc.tensor.matmul(pR[:, i, :], lhsT=OT[:, t0 + i, :],
                                 rhs=xrt[:, b, :],
                                 start=(i == 0), stop=(i == gsz - 1))
            nc.scalar.activation(out=Rs[:, t0:t0 + gsz, :],
                                 in_=pR[:, 0:gsz, :], func=AF.Copy)
        st[b] = dict(G=G, OT=OT, Rs=Rs, pos=pos, Tb=Tb)

    def stage2(b):
        """Logits + softmax weights (DVE + ACT)."""
        s = st[b]
        G, Rs, pos, Tb = s["G"], s["Rs"], s["pos"], s["Tb"]
        dcol = dcl[:, pos // P:(pos + Tb * P) // P, :]   # [P, Tb, 1]
        # z = xl'[src] + xr'[dst]  (att-scaled space)
        Z = wp.tile([P, Tmax, P], dt, tag="Z", name="Z")
        nc.vector.tensor_tensor(out=Z[:, 0:Tb, :], in0=G[:, 0:Tb, :],
                                in1=Rs[:, 0:Tb, :], op=ALU.add)
        # signed-att leaky-relu: positive-att cols get lrelu(z) = max(z,.2z),
        # negative-att cols get min(z,.2z) = Prelu(alpha=1/s)(s*z), s=.2 --
        # then the plain row-sum per layer IS the logit.
        U = wp.tile([P, Tmax, P], dt, tag="U", name="U")
        for l in range(n_lay):
            k = kpos[l]
            c0 = l * feat
            if k > 0:
                nc.scalar.activation(out=U[:, 0:Tb, c0:c0 + k],
                                     in_=Z[:, 0:Tb, c0:c0 + k],
                                     func=AF.Prelu, alpha=SLOPE)
            if k < feat:
                nc.scalar.activation(out=U[:, 0:Tb, c0 + k:c0 + feat],
                                     in_=Z[:, 0:Tb, c0 + k:c0 + feat],
                                     func=AF.Prelu, alpha=1.0 / SLOPE,
                                     scale=SLOPE)
        e = wp.tile([P, Tmax, n_lay, 1], F32, tag="e", name="e")
        nc.vector.tensor_reduce(
            out=e[:, 0:Tb, :, :],
            in_=U[:, 0:Tb, :].rearrange("p t (l f) -> p t l f", l=n_lay),
            axis=mybir.AxisListType.X, op=ALU.add)
        # clamp: skipped pad slots carry stale-but-finite data whose logit
        # can be large; exp must stay finite so 0*exp(e) == 0 in the matmul
        nc.vector.tensor_scalar(out=e[:, 0:Tb, :, :], in0=e[:, 0:Tb, :, :],
                                scalar1=ECLAMP, scalar2=None, op0=ALU.min)
        ee = wp.tile([P, Tmax, n_lay, 1], dt, tag="ee", name="ee")
        nc.scalar.activation(out=ee[:, 0:Tb, :, :], in_=e[:, 0:Tb, :, :],
                             func=AF.Exp)

        # one-hot (dst-col) and softmax-weighted source rows
        O = wp.tile([P, Tmax, 1, P], dt, tag="O", name="O")
        nc.vector.tensor_tensor(
            out=O[:, 0:Tb, 0, :],
            in0=dcol.to_broadcast([P, Tb, P]),
            in1=iotaRep_sb[:, 0:Tb, :], op=ALU.is_equal)
        # W_l = ee_l * G_l: per-layer halves, so the weighting pass covers
        # 128 cols total regardless of n_lay (the one-hot O is shared)
        W = wp.tile([P, Tmax, P], dt, tag="W", name="W")
        Wv = W[:, 0:Tb, :].rearrange("p t (l f) -> p t l f", l=n_lay)
        nc.vector.tensor_tensor(
            out=Wv[:],
            in0=G[:, 0:Tb, :].rearrange("p t (l f) -> p t l f", l=n_lay),
            in1=ee[:, 0:Tb, :, :].to_broadcast([P, Tb, n_lay, feat]),
            op=ALU.mult)
        s.update(O=O, W=W, ee=ee)

    def stage3(b):
        """Accumulate numerator/denominator (PE) and finalize the bin."""
        s = st.pop(b)
        O, W, ee, Tb = s["O"], s["W"], s["ee"], s["Tb"]
        # numerator cols [0:feat], denominator col [feat]: one PSUM bank per
        # layer.  num's start=True zeroes the whole bank, so the den matmul
        # always runs start=False and lands on a zeroed column.  All matmuls
        # of a tile share lhsT = the one-hot O_t.
        pn = [pp.tile([P, feat + n_lay if l == 0 else feat], F32,
                      tag=f"pn{l}", name=f"pn{l}")
              for l in range(n_lay)]
        for t in range(Tb):
            for l in range(n_lay):
                nc.tensor.matmul(
                    pn[l][:, 0:feat], lhsT=O[:, t, 0, :],
                    rhs=W[:, t, l * feat:(l + 1) * feat],
                    start=(t == 0), stop=(t == Tb - 1))
            nc.tensor.matmul(
                pn[0][:, feat:feat + n_lay], lhsT=O[:, t, 0, :],
                rhs=ee[:, t, :, 0], start=False, stop=(t == Tb - 1))
        finalize(b, pn)

    # 1-bin software pipeline: keep the TensorEngine fed (its stream is
    # in-order, so bin b+1's xr matmuls must be emitted before bin b's
    # accumulation matmuls to overlap with the DVE/ACT middle stages).
    stage1(0)
    for b in range(cfg.nbins):
        if b + 1 < cfg.nbins:
            stage1(b + 1)
        stage2(b)
        stage3(b)


def _load_consts(nc, cp, names_shapes):
    out = []
    for name, shape, dt in names_shapes:
        dram = nc.dram_tensor(name, shape, dt, kind="ExternalInput")
        sb = cp.tile(shape, dt, name=name + "_sb")
        nc.sync.dma_start(sb[:], dram[:])
        out.append(sb)
    return out


def build_edge1(cfg: Cfg, pr, kpos, dt=F32, nq=1):
    """Edge phase of layer 1 + dense transforms of layers 2/3."""
    nc = _new_nc(cfg, nq)
    hid, out, nlp, nbins = cfg.hid, cfg.out, cfg.nlp, cfg.nbins
    nslots = pr["nslots"]
    Tmax = int(pr["tbin"].max())
    XLchunks = [nc.dram_tensor(f"XL1c{o}", [cfg.chrows, hid], dt,
                               kind="ExternalInput") for o in range(cfg.nchunk)]
    XR = nc.dram_tensor("XR1", [nlp, hid], dt, kind="ExternalInput")
    GIDX = nc.dram_tensor("gidx", [P, nslots // 16], I16, kind="ExternalInput")
    DCOL = nc.dram_tensor("dcol", [P, nslots // 128, 1], dt, kind="ExternalInput")
    OHT = nc.dram_tensor("oht", [P, nslots], dt, kind="ExternalInput")
    XL23 = nc.dram_tensor("XL23", [nlp, P], dt, kind="ExternalOutput")
    XR23 = nc.dram_tensor("XR23", [nlp, P], dt, kind="ExternalOutput")

    with tile.TileContext(nc) as tc:
        with tc.tile_pool(name="const", bufs=1) as cp, \
             tc.tile_pool(name="gath", bufs=3) as gp, \
             tc.tile_pool(name="work", bufs=2) as wp, \
             tc.tile_pool(name="psum", bufs=2, space="PSUM") as pp, \
             tc.tile_pool(name="psfin", bufs=2, space="PSUM") as pf:
            (iotaRep_sb, winv1B_sb, b1B_sb, ident_sb, w23lr_sb,
             b23lr_sb) = _load_consts(nc, cp, [
                 ("iotaRep", [P, Tmax, P], dt),
                 ("winv1B", [P, hid], F32),
                 ("b1B", [P, hid], F32),
                 ("identB", [P, P], F32),
                 ("w23lr", [hid, 2 * P], F32),
                 ("b23lrB", [P, 2 * P], F32)])

            def finalize(b, pn):
                ms = slice(b * P, (b + 1) * P)
                d = wp.tile([P, 1], F32, tag="d", name="d")
                nc.vector.tensor_scalar_add(d[:], pn[0][:, hid:hid + 1], EPS)
                r = wp.tile([P, 1], F32, tag="r", name="r")
                nc.vector.reciprocal(r[:], d[:])
                h = wp.tile([P, hid], F32, tag="h", name="h")
                # h = (num * 1/den) * winv  (unscale att, permuted order)
                nc.vector.scalar_tensor_tensor(
                    out=h[:], in0=pn[0][:, 0:hid], scalar=r[:],
                    in1=winv1B_sb[:], op0=ALU.mult, op1=ALU.mult)
                nc.vector.tensor_tensor(out=h[:], in0=h[:], in1=b1B_sb[:], op=ALU.add)
                nc.scalar.activation(out=h[:], in_=h[:], func=AF.Relu)
                pst = pf.tile([P, P], F32, tag="pst", name="pst")
                nc.tensor.transpose(out=pst[:], in_=h[:], identity=ident_sb[:])
                hT = wp.tile([P, P], F32, tag="hT", name="hT")
                nc.scalar.activation(out=hT[:], in_=pst[:], func=AF.Copy)
                psB = pf.tile([P, 2 * P], F32, tag="psB", name="psB")
                nc.tensor.matmul(psB[:], lhsT=hT[:, 0:hid], rhs=w23lr_sb[:],
                                 start=True, stop=True)
                olr = wp.tile([P, 2 * P], dt, tag="olr", name="olr")
                nc.vector.tensor_tensor(out=olr[:], in0=psB[:], in1=b23lr_sb[:],
                                        op=ALU.add)
                nc.sync.dma_start(XL23[ms, :], olr[:, 0:P])
                nc.sync.dma_start(XR23[ms, :], olr[:, P:2 * P])

            _edge_phase(nc, tc, cfg, pr, (cp, gp, wp, pp),
                        (XLchunks, XR, GIDX, DCOL, OHT),
                        iotaRep_sb, 1, kpos, finalize, dt)
    nc.compile()
    return nc


def build_edge23(cfg: Cfg, pr, kpos, dt=F32, nq=1):
    """Edge phases of layers 2 and 3 (shared gather)."""
    nc = _new_nc(cfg, nq)
    out, nlp, nbins = cfg.out, cfg.nlp, cfg.nbins
    nslots = pr["nslots"]
    Tmax = int(pr["tbin"].max())
    XLchunks = [nc.dram_tensor(f"XL23c{o}", [cfg.chrows, P], dt,
                               kind="ExternalInput") for o in range(cfg.nchunk)]
    XR = nc.dram_tensor("XR23", [nlp, P], dt, kind="ExternalInput")
    GIDX = nc.dram_tensor("gidx", [P, nslots // 16], I16, kind="ExternalInput")
    DCOL = nc.dram_tensor("dcol", [P, nslots // 128, 1], dt, kind="ExternalInput")
    OHT = nc.dram_tensor("oht", [P, nslots], dt, kind="ExternalInput")
    MU = nc.dram_tensor("MU", [nlp, out], F32, kind="ExternalOutput")
    LV = nc.dram_tensor("LV", [nlp, out], F32, kind="ExternalOutput")

    with tile.TileContext(nc) as tc:
        with tc.tile_pool(name="const", bufs=1) as cp, \
             tc.tile_pool(name="gath", bufs=3) as gp, \
             tc.tile_pool(name="work", bufs=2) as wp, \
             tc.tile_pool(name="psum", bufs=2, space="PSUM") as pp:
            (iotaRep_sb, winvmuB_sb, winvlvB_sb, bmu_sb, blv_sb) = _load_consts(
                nc, cp, [
                    ("iotaRep", [P, Tmax, P], dt),
                    ("winvmuB", [P, out], F32),
                    ("winvlvB", [P, out], F32),
                    ("bmuB", [P, out], F32),
                    ("blvB", [P, out], F32)])

            def finalize(b, pn):
                ms = slice(b * P, (b + 1) * P)
                for l, (winv, bias, dest, tg) in enumerate(
                        ((winvmuB_sb, bmu_sb, MU, "mu"),
                         (winvlvB_sb, blv_sb, LV, "lv"))):
                    d = wp.tile([P, 1], F32, tag=f"d{tg}", name="d")
                    nc.vector.tensor_scalar_add(d[:], pn[0][:, out + l:out + l + 1],
                                                EPS)
                    r = wp.tile([P, 1], F32, tag=f"r{tg}", name="r")
                    nc.vector.reciprocal(r[:], d[:])
                    o = wp.tile([P, out], F32, tag=f"o{tg}", name="o")
                    nc.vector.scalar_tensor_tensor(
                        out=o[:], in0=pn[l][:, 0:out], scalar=r[:],
                        in1=winv[:], op0=ALU.mult, op1=ALU.mult)
                    nc.vector.tensor_tensor(out=o[:], in0=o[:], in1=bias[:], op=ALU.add)
                    nc.sync.dma_start(dest[ms, :], o[:])

            _edge_phase(nc, tc, cfg, pr, (cp, gp, wp, pp),
                        (XLchunks, XR, GIDX, DCOL, OHT),
                        iotaRep_sb, 2, kpos, finalize, dt)
    nc.compile()
    return nc


# ----------------------------------------------------------------------------
# Host orchestration
# ----------------------------------------------------------------------------

def _bb(v, rows=P):
    """Broadcast a 1-D row vector to [rows, len] f32."""
    v = np.asarray(v, np.float32).reshape(1, -1)
    return np.ascontiguousarray(np.broadcast_to(v, (rows, v.shape[1])))


def _hw_runner(nc, in_maps, cfg, trace=False):
    from concourse import bass_utils
    r = bass_utils.run_bass_kernel_spmd(
        nc, in_maps, core_ids=list(range(cfg.ncores)), trace=trace)
    return r.results, r.exec_time_ns


class _State:
    """Cached compiled programs + prep, keyed by edge structure."""
    key = None
    progs = None
    prep = None
    fold = None


EDT = BF16 if not int(os.environ.get("GAT_F32", "0")) else F32
NQUEUES = 4


def fold_weights(w):
    """Fold att into the tables: scale columns by att (signed), permute so
    positive-att columns come first in each layer block.  Returns folded
    weights + the inverse data needed at finalize / host postprocess."""
    f = {}
    perms, kpos = {}, {}
    for name, fo in (("sh", HID), ("mu", OUT), ("lv", OUT)):
        a = np.asarray(w[f"{name}_att"], np.float32).reshape(-1)  # [fo]
        pi = np.argsort(a <= 0, kind="stable")  # positive cols first
        perms[name] = pi
        kpos[name] = int((a > 0).sum())
        f[f"{name}_a"] = np.ascontiguousarray(a[pi])  # signed att, permuted
        f[f"{name}_Wl"] = np.ascontiguousarray(
            (np.asarray(w[f"{name}_Wl"], np.float32) * a)[:, pi])
        f[f"{name}_Wr"] = np.ascontiguousarray(
            (np.asarray(w[f"{name}_Wr"], np.float32) * a)[:, pi])
        f[f"{name}_bl"] = (np.asarray(w[f"{name}_bl"], np.float32) * a)[pi]
        f[f"{name}_br"] = (np.asarray(w[f"{name}_br"], np.float32) * a)[pi]
        f[f"{name}_winv"] = 1.0 / f[f"{name}_a"]
        f[f"{name}_b"] = np.asarray(w[f"{name}_b"], np.float32)[pi]
    # layer-1 permutation also permutes h's columns -> permute W23 rows
    pi1 = perms["sh"]
    f["mu_Wl"] = np.ascontiguousarray(f["mu_Wl"][pi1])
    f["mu_Wr"] = np.ascontiguousarray(f["mu_Wr"][pi1])
    f["lv_Wl"] = np.ascontiguousarray(f["lv_Wl"][pi1])
    f["lv_Wr"] = np.ascontiguousarray(f["lv_Wr"][pi1])
    f["perms"] = perms
    f["kpos"] = kpos
    return f


def build_progs(cfg, pr, fold, dt=None, nq=None):
    dt = EDT if dt is None else dt
    nq = NQUEUES if nq is None else nq
    kp = fold["kpos"]
    return dict(
        dense1=build_dense1(cfg, dt),
        edge1=build_edge1(cfg, pr, [kp["sh"]], dt, nq),
        edge23=build_edge23(cfg, pr, [kp["mu"], kp["lv"]], dt, nq),
    )


def forward(cfg, x, ei_unused, w, f, pr, progs, runner, dt=None):
    dt = EDT if dt is None else dt
    ndt = mybir.dt.np(dt)
    perm = pr["perm"]                    # [ncores, nlp] node ids or -1
    Tmax = int(pr["tbin"].max())
    profile = {}
    dcol_in = [np.ascontiguousarray(pr["dstcol"][c][:, :, None].astype(ndt))
               for c in range(cfg.ncores)]
    oht_in = [np.ascontiguousarray(pr["onehotT"][c].astype(ndt))
              for c in range(cfg.ncores)]

    hid, out, nlp, ntab = cfg.hid, cfg.out, cfg.nlp, cfg.ntab

    # ---- launch A: dense1 (att-scaled, sign-permuted weights) -------------
    in_maps = []
    for c in range(cfg.ncores):
        xs = np.zeros((nlp, cfg.fin), np.float32)
        sel = perm[c] >= 0
        xs[sel] = x[perm[c][sel]]
        in_maps.append(dict(
            xT=np.ascontiguousarray(xs.T), wl=f["sh_Wl"], wr=f["sh_Wr"],
            blB=_bb(f["sh_bl"]), brB=_bb(f["sh_br"])))
    rA, profile["A"] = runner(progs["dense1"], in_maps, cfg)
    XL1full = np.concatenate([rA[c]["XL1"] for c in range(cfg.ncores)])
    XL1ch = {f"XL1c{o}": np.ascontiguousarray(
        XL1full[o * cfg.chrows:(o + 1) * cfg.chrows])
        for o in range(cfg.nchunk)}
    XR1 = [rA[c]["XR1"] for c in range(cfg.ncores)]

    # ---- launch B: edge1 + dense23 ----------------------------------------
    iotaRep = np.ascontiguousarray(np.broadcast_to(
        np.arange(P, dtype=np.float32), (P, Tmax, P))).astype(ndt)
    w23l = np.concatenate([f["mu_Wl"], f["lv_Wl"]], axis=1)
    w23r = np.concatenate([f["mu_Wr"], f["lv_Wr"]], axis=1)
    b23l = np.concatenate([f["mu_bl"], f["lv_bl"]])
    b23r = np.concatenate([f["mu_br"], f["lv_br"]])
    ident = np.eye(P, dtype=np.float32)
    in_maps = []
    for c in range(cfg.ncores):
        in_maps.append(dict(
            XR1=XR1[c], **XL1ch,
            gidx=pr["gidx16"][c], dcol=dcol_in[c], oht=oht_in[c],
            iotaRep=iotaRep, winv1B=_bb(f["sh_winv"]), b1B=_bb(f["sh_b"]),
            identB=ident,
            w23lr=np.ascontiguousarray(np.concatenate([w23l, w23r], axis=1)),
            b23lrB=_bb(np.concatenate([b23l, b23r]))))
    rB, profile["B"] = runner(progs["edge1"], in_maps, cfg)
    XL23full = np.concatenate([rB[c]["XL23"] for c in range(cfg.ncores)])
    XL23ch = {f"XL23c{o}": np.ascontiguousarray(
        XL23full[o * cfg.chrows:(o + 1) * cfg.chrows])
        for o in range(cfg.nchunk)}
    XR23 = [rB[c]["XR23"] for c in range(cfg.ncores)]

    # ---- launch C: edge23 --------------------------------------------------
    in_maps = []
    for c in range(cfg.ncores):
        in_maps.append(dict(
            XR23=XR23[c], **XL23ch,
            gidx=pr["gidx16"][c], dcol=dcol_in[c], oht=oht_in[c],
            iotaRep=iotaRep,
            winvmuB=_bb(f["mu_winv"]), winvlvB=_bb(f["lv_winv"]),
            bmuB=_bb(f["mu_b"]), blvB=_bb(f["lv_b"])))
    rC, profile["C"] = runner(progs["edge23"], in_maps, cfg)

    MU = np.concatenate([rC[c]["MU"] for c in range(cfg.ncores)])
    LV = np.concatenate([rC[c]["LV"] for c in range(cfg.ncores)])
    # undo the sign permutation of the output columns
    mu_u = np.empty_like(MU)
    mu_u[:, f["perms"]["mu"]] = MU
    lv_u = np.empty_like(LV)
    lv_u[:, f["perms"]["lv"]] = LV
    mu = mu_u[pr["slot_global"]]
    lv = lv_u[pr["slot_global"]]
    return (mu, lv), profile


def kernel(**inputs):
    cfg = Cfg()
    x = np.asarray(inputs["x"], np.float32)
    ei = np.asarray(inputs["edge_index"]).astype(np.int64)
    w = {k: np.asarray(v, np.float32) for k, v in inputs.items()
         if k not in ("x", "edge_index")}

    fold = fold_weights(w)
    key = (hash(ei.tobytes()), tuple(sorted(fold["kpos"].items())))
    if _State.key != key:
        pr = prep_graph(cfg, ei)
        _State.prep = pr
        _State.progs = build_progs(cfg, pr, fold)
        _State.key = key
    _State.fold = fold

    trace = bool(int(os.environ.get("GAT_TRACE", "0")))
    runner = functools.partial(_hw_runner, trace=trace)
    (mu, lv), profile = forward(cfg, x, ei, w, _State.fold, _State.prep,
                                _State.progs, runner)
    kernel._last_profile = profile
    return (mu, lv)


kernel._last_profile = None


# revision 22
# speedup vs baseline: 1.5580x; 1.0027x over previous
# GATv2 encoder (3x GATv2Conv, H=1) on 8 Trainium2 NeuronCores.
#
# Sharding: nodes partitioned by dst across 8 cores (graph parallel).
# Edge work per core is organized as 98 "bins" of <=128 dst nodes each
# (host-side bin-packing balances edge counts); a bin's edges are grouped
# by source-table chunk (4 chunks of 25088 rows so indices fit int16) and
# padded to 128-edge tiles.  Per bin: batched dma_gather of att-scaled
# source rows xl'[src] (trailing -1 indices skip pad descriptors), xr[dst]
# reconstructed on the TensorEngine from the SBUF-resident local XR table
# via a host-baked one-hot (OT), leaky-relu logits on the ScalarEngine
# (att folded into the tables; columns permuted by sign(att) host-side),
# and per-tile one-hot matmuls accumulating softmax numerator + denominator
# in one PSUM bank.  Host gathers per-core dense outputs between launches.
import os
import sys
import math
import functools
import numpy as np

for _p in ("/opt/trn_rl_repo",):
    if _p not in sys.path and os.path.isdir(_p):
        sys.path.insert(0, _p)

import concourse.bass as bass
import concourse.mybir as mybir
import concourse.tile as tile
from concourse import bacc

F32 = mybir.dt.float32
BF16 = mybir.dt.bfloat16
I16 = mybir.dt.int16
I32 = mybir.dt.int32
AF = mybir.ActivationFunctionType
ALU = mybir.AluOpType

# Problem constants (hardcoded per contract)
N = 100_000
E = 1_600_000
IN, HID, OUT, H = 256, 128, 64, 1
SLOPE = 0.2
NCORES = 8
P = 128
EPS = 1e-30
ECLAMP = 30.0       # logit clamp: keeps exp() finite on stale pad slots
NCHUNK = 4          # source-table chunks (rows per chunk must fit int16)


class Cfg:
    """Geometry, parameterized so small test instances can be built."""

    def __init__(self, n=N, e=E, fin=IN, hid=HID, out=OUT, ncores=NCORES):
        self.n, self.e, self.fin, self.hid, self.out = n, e, fin, hid, out
        self.ncores = ncores
        assert n % ncores == 0
        self.nl = n // ncores                  # dst nodes per core
        self.nbins = math.ceil(self.nl / P)    # bins per core
        self.nlp = self.nbins * P              # padded local nodes
        self.ntab = self.nlp * ncores          # rows in gathered tables
        self.nchunk = min(NCHUNK, ncores)
        assert self.ntab % self.nchunk == 0
        self.chrows = self.ntab // self.nchunk  # rows per source chunk
        assert self.chrows < 32768, "chunk rows must fit int16"
        assert fin % P == 0
        self.kt = fin // P                     # K-tiles for dense1


# ----------------------------------------------------------------------------
# Host-side graph preprocessing
# ----------------------------------------------------------------------------

def prep_graph(cfg: Cfg, edge_index: np.ndarray):
    """Bin-pack dsts, group edges by (bin, src chunk), build index arrays."""
    n, ncores, nl, nbins, nlp = cfg.n, cfg.ncores, cfg.nl, cfg.nbins, cfg.nlp
    nck, chrows = cfg.nchunk, cfg.chrows
    src = np.concatenate([edge_index[0], np.arange(n, dtype=np.int64)])
    dst = np.concatenate([edge_index[1], np.arange(n, dtype=np.int64)])

    # --- per-core bin-packing of dst nodes ---------------------------------
    slot_global = np.full(n, -1, dtype=np.int64)  # node -> row in table space
    deg_all = np.bincount(dst, minlength=n)
    import heapq
    for c in range(ncores):
        lo, hi = c * nl, (c + 1) * nl
        deg = deg_all[lo:hi]
        order = np.argsort(-deg, kind="stable")
        heap = [(0, 0, b) for b in range(nbins)]
        heapq.heapify(heap)
        stash = []
        for node in order:
            d = int(deg[node])
            while True:
                s, cnt, b = heapq.heappop(heap)
                if cnt < P:
                    break
                stash.append((s, cnt, b))
            slot_global[lo + node] = c * nlp + b * P + cnt
            heapq.heappush(heap, (s + d, cnt + 1, b))
            for it in stash:
                heapq.heappush(heap, it)
            stash.clear()

    # --- group edges by (core, bin, chunk) ---------------------------------
    sslot = slot_global[src]
    dslot = slot_global[dst]
    chunk = sslot // chrows
    binid = dslot // P                    # global bin id = core*nbins + bin
    key = binid * nck + chunk
    order = np.argsort(key, kind="stable")
    s_o, d_o, k_o = sslot[order], dslot[order], key[order]
    nkeys = ncores * nbins * nck
    cnts = np.bincount(k_o, minlength=nkeys).reshape(ncores, nbins, nck)
    offs = np.concatenate([[0], np.cumsum(cnts.reshape(-1))])

    # uniform-across-cores tiles per (bin, chunk)
    tbo = np.maximum(np.ceil(cnts / P).astype(np.int64).max(axis=0),
                     (cnts.max(axis=0) > 0))  # [nbins, nck]
    tbin = tbo.sum(axis=1)                 # [nbins] tiles per bin
    nslots = int(tbin.sum()) * P           # edge slots per core
    # uniform valid-index count per (bin, chunk): max across cores; slots
    # beyond it carry -1 indices, which the gather ucode skips (trailing
    # negatives emit no DMA descriptors).
    vcnt = np.maximum(cnts.max(axis=0), (tbo > 0)).astype(np.int64)

    # --- per-core index arrays ---------------------------------------------
    # gidx16: wrapped-16 int16 chunk-local src indices, [128, nslots//16]
    # dstcol: lane-major one-hot columns (f32),         [128, nslots//128]
    # onehotT: OT[d, slot] = 1 iff slot's dst row (bin-local) == d
    gidx16 = np.zeros((ncores, 128, nslots // 16), np.int16)
    dstcol = np.full((ncores, 128, nslots // 128), 200.0, np.float32)
    onehotT = np.zeros((ncores, 128, nslots), np.float32)
    for c in range(ncores):
        pos = 0  # slot position within the core's stream
        for b in range(nbins):
            rbase = b * P
            for o in range(nck):
                kk = int(cnts[c, b, o])
                so = offs[(c * nbins + b) * nck + o]
                slots = int(tbo[b, o]) * P
                if slots == 0:
                    continue
                j = np.arange(kk)
                jp = pos + j
                vc = int(vcnt[b, o])
                # [0:kk] real, [kk:vc] index-0 pads (uniform descriptor
                # count across cores), [vc:slots] -1 (skipped by ucode)
                g = np.full(slots, -1, np.int16)
                g[j] = (s_o[so:so + kk] - o * chrows).astype(np.int16)
                g[kk:vc] = 0
                jj = pos + np.arange(slots)
                gidx16[c, jj % 16, jj // 16] = g
                dloc = (d_o[so:so + kk] - (c * nlp + rbase)).astype(np.int64)
                dstcol[c, jp % 128, jp // 128] = dloc.astype(np.float32)
                onehotT[c, dloc, jp] = 1.0
                pos += slots
        assert pos == nslots
        # the Q7 gather ucode reads indices from its own 16-partition group:
        # replicate the wrapped-16 data across all 8 groups
        gidx16[c] = np.tile(gidx16[c, :16], (8, 1))

    # node permutation per core: slot s -> original node (or -1)
    perm = np.full((ncores, nlp), -1, dtype=np.int64)
    nodes = np.where(slot_global >= 0)[0]
    perm.reshape(-1)[slot_global[nodes]] = nodes

    return dict(
        tbo=tbo, tbin=tbin, vcnt=vcnt, nslots=nslots,
        slot_global=slot_global, perm=perm,
        gidx16=gidx16, dstcol=dstcol, onehotT=onehotT,
    )


# ----------------------------------------------------------------------------
# Device program builders (single SPMD program, data differs per core)
# ----------------------------------------------------------------------------

def _new_nc(cfg, nq=1):
    return bacc.Bacc("TRN2", target_bir_lowering=False, debug=False,
                     enable_asserts=False, num_devices=cfg.ncores,
                     num_swdge_queues=nq)


def build_dense1(cfg: Cfg, dt=F32):
    """xT [fin, nlp] -> XL1 [nlp, hid], XR1 [nlp, hid]."""
    nc = _new_nc(cfg)
    fin, hid, nlp, kt = cfg.fin, cfg.hid, cfg.nlp, cfg.kt
    xT = nc.dram_tensor("xT", [fin, nlp], F32, kind="ExternalInput")
    wl = nc.dram_tensor("wl", [fin, hid], F32, kind="ExternalInput")
    wr = nc.dram_tensor("wr", [fin, hid], F32, kind="ExternalInput")
    blB = nc.dram_tensor("blB", [P, hid], F32, kind="ExternalInput")
    brB = nc.dram_tensor("brB", [P, hid], F32, kind="ExternalInput")
    XL = nc.dram_tensor("XL1", [nlp, hid], dt, kind="ExternalOutput")
    XR = nc.dram_tensor("XR1", [nlp, hid], dt, kind="ExternalOutput")

    mtiles = nlp // P
    with tile.TileContext(nc) as tc:
        with tc.tile_pool(name="const", bufs=1) as cp, \
             tc.tile_pool(name="work", bufs=4) as wp, \
             tc.tile_pool(name="psum", bufs=4, space="PSUM") as pp:
            xk = cp.tile([P, kt, nlp], F32)
            nc.sync.dma_start(xk[:], xT[:].rearrange("(k p) n -> p k n", p=P))
            wl_sb = cp.tile([P, kt, hid], F32)
            nc.sync.dma_start(wl_sb[:], wl[:].rearrange("(k p) h -> p k h", p=P))
            wr_sb = cp.tile([P, kt, hid], F32)
            nc.sync.dma_start(wr_sb[:], wr[:].rearrange("(k p) h -> p k h", p=P))
            blB_sb = cp.tile([P, hid], F32)
            nc.sync.dma_start(blB_sb[:], blB[:])
            brB_sb = cp.tile([P, hid], F32)
            nc.sync.dma_start(brB_sb[:], brB[:])

            for m in range(mtiles):
                ms = slice(m * P, (m + 1) * P)
                psl = pp.tile([P, hid], F32, tag="psl")
                psr = pp.tile([P, hid], F32, tag="psr")
                for k in range(kt):
                    nc.tensor.matmul(psl[:], lhsT=xk[:, k, ms], rhs=wl_sb[:, k, :],
                                     start=(k == 0), stop=(k == kt - 1))
                for k in range(kt):
                    nc.tensor.matmul(psr[:], lhsT=xk[:, k, ms], rhs=wr_sb[:, k, :],
                                     start=(k == 0), stop=(k == kt - 1))
                ol = wp.tile([P, hid], dt, tag="ol")
                nc.vector.tensor_tensor(out=ol[:], in0=psl[:], in1=blB_sb[:], op=ALU.add)
                orr = wp.tile([P, hid], dt, tag="orr")
                nc.vector.tensor_tensor(out=orr[:], in0=psr[:], in1=brB_sb[:], op=ALU.add)
                nc.sync.dma_start(XL[ms, :], ol[:])
                nc.sync.dma_start(XR[ms, :], orr[:])
    nc.compile()
    return nc


def _edge_phase(nc, tc, cfg, pr, pools, tabs, consts, n_lay, kpos, finalize, dt):
    """Shared edge pipeline over bins (att folded into the tables).

    Gathered rows are 128 wide: n_lay layer blocks of feat=128//n_lay cols;
    within each block, columns with positive att come first (kpos[l] of
    them), as permuted host-side.  xr[dst] is reconstructed per tile on the
    TensorEngine from the SBUF-resident XR table via the host-baked one-hot
    OT.  finalize(b, psums): consume accumulated PSUM tiles per bin
    (cols [0:feat] numerator, col [feat] denominator).
    """
    from concourse import library_config
    nc.gpsimd.load_library(library_config.mlp)
    cp, gp, wp, pp = pools
    XLchunks, XR, GIDX, DCOL, OHT = tabs
    iotaRep_sb = consts
    tbo, tbin, vcnt = pr["tbo"], pr["tbin"], pr["vcnt"]
    nslots = pr["nslots"]
    nck, chrows = cfg.nchunk, cfg.chrows
    feat = P // n_lay
    Tmax = int(tbin.max())
    NQ = nc.num_swdge_queues
    qn = 0

    # whole-launch index array resident in SBUF
    gix = cp.tile([P, nslots // 16], I16)
    nc.sync.dma_start(gix[:], GIDX[:])
    dcl = cp.tile([P, nslots // 128, 1], dt)
    nc.sync.dma_start(dcl[:], DCOL[:])
    # local XR table resident: [p, bin, hid] with p = row within bin
    xrt = cp.tile([P, cfg.nbins, P], dt)
    nc.sync.dma_start(xrt[:], XR[:].rearrange("(b p) h -> p b h", p=P))

    bpos = np.concatenate([[0], np.cumsum(tbin)]) * P
    st = {}

    def stage1(b):
        """Gathers + OT stream + xr reconstruction (PE matmul, ACT copy)."""
        nonlocal qn
        pos = int(bpos[b])
        Tb = int(tbin[b])
        G = gp.tile([P, Tmax, P], dt, tag="G", name="G")
        OT = gp.tile([P, Tmax, P], dt, tag="OT", name="OT")
        Rs = gp.tile([P, Tmax, P], dt, tag="Rs", name="Rs")
        if b < 3:
            # first use of each gather-pool buffer (bufs=3): clear so skipped
            # (-1) pad slots hold finite values, not uninitialized SBUF bits
            nc.vector.memset(G[:], 0.0)
        to = 0
        for o in range(nck):
            tt = int(tbo[b, o])
            if tt == 0:
                continue
            nidx = tt * P
            vc = int(vcnt[b, o])
            col = (pos + to * P) // 16
            nc.gpsimd.dma_gather(
                out_ap=G[:, to:to + tt, :],
                in_ap=XLchunks[o][:],
                idxs_ap=gix[:, col:col + nidx // 16],
                num_idxs=nidx, num_idxs_reg=vc, elem_size=P,
                single_packet=(nidx <= 1024), queue_num=qn % NQ)
            qn += 1
            to += tt
        assert to == Tb
        nc.sync.dma_start(OT[:, 0:Tb, :],
                          OHT[:, pos:pos + Tb * P].rearrange(
                              "d (t e) -> d t e", e=P))
        # xr[dst] per edge: R_t = OT_t^T @ XR_bin on the TensorEngine.
        # 4 tiles share one PSUM bank (slice 0's start=True zeroes the whole
        # bank; later slices accumulate onto zeroed regions), so one ACT
        # copy moves 4 tiles to SBUF (bf16) and the z-add runs 2x batched.
        for t0 in range(0, Tb, 4):
            gsz = min(4, Tb - t0)
            pR = pp.tile([P, 4, P], F32, tag="pR", name="pR")
            for i in range(gsz):
                nc.tensor.matmul(pR[:, i, :], lhsT=OT[:, t0 + i, :],
                                 rhs=xrt[:, b, :],
                                 start=(i == 0), stop=(i == gsz - 1))
            nc.scalar.activation(out=Rs[:, t0:t0 + gsz, :],
                                 in_=pR[:, 0:gsz, :], func=AF.Copy)
        st[b] = dict(G=G, OT=OT, Rs=Rs, pos=pos, Tb=Tb)

    def stage2(b):
        """Logits + softmax weights (DVE + ACT)."""
        s = st[b]
        G, Rs, pos, Tb = s["G"], s["Rs"], s["pos"], s["Tb"]
        dcol = dcl[:, pos // P:(pos + Tb * P) // P, :]   # [P, Tb, 1]
        # z = xl'[src] + xr'[dst]  (att-scaled space)
        Z = wp.tile([P, Tmax, P], dt, tag="Z", name="Z")
        nc.vector.tensor_tensor(out=Z[:, 0:Tb, :], in0=G[:, 0:Tb, :],
                                in1=Rs[:, 0:Tb, :], op=ALU.add)
        # signed-att leaky-relu: positive-att cols get lrelu(z) = max(z,.2z),
        # negative-att cols get min(z,.2z) = Prelu(alpha=1/s)(s*z), s=.2 --
        # then the plain row-sum per layer IS the logit.
        U = wp.tile([P, Tmax, P], dt, tag="U", name="U")
        for l in range(n_lay):
            k = kpos[l]
            c0 = l * feat
            if k > 0:
                nc.scalar.activation(out=U[:, 0:Tb, c0:c0 + k],
                                     in_=Z[:, 0:Tb, c0:c0 + k],
                                     func=AF.Prelu, alpha=SLOPE)
            if k < feat:
                nc.scalar.activation(out=U[:, 0:Tb, c0 + k:c0 + feat],
                                     in_=Z[:, 0:Tb, c0 + k:c0 + feat],
                                     func=AF.Prelu, alpha=1.0 / SLOPE,
                                     scale=SLOPE)
        e = wp.tile([P, Tmax, n_lay, 1], F32, tag="e", name="e")
        nc.vector.tensor_reduce(
            out=e[:, 0:Tb, :, :],
            in_=U[:, 0:Tb, :].rearrange("p t (l f) -> p t l f", l=n_lay),
            axis=mybir.AxisListType.X, op=ALU.add)
        # clamp: skipped pad slots carry stale-but-finite data whose logit
        # can be large; exp must stay finite so 0*exp(e) == 0 in the matmul
        nc.vector.tensor_scalar(out=e[:, 0:Tb, :, :], in0=e[:, 0:Tb, :, :],
                                scalar1=ECLAMP, scalar2=None, op0=ALU.min)
        ee = wp.tile([P, Tmax, n_lay, 1], dt, tag="ee", name="ee")
        nc.scalar.activation(out=ee[:, 0:Tb, :, :], in_=e[:, 0:Tb, :, :],
                             func=AF.Exp)

        # one-hot (dst-col) and softmax-weighted source rows
        O = wp.tile([P, Tmax, 1, P], dt, tag="O", name="O")
        nc.vector.tensor_tensor(
            out=O[:, 0:Tb, 0, :],
            in0=dcol.to_broadcast([P, Tb, P]),
            in1=iotaRep_sb[:, 0:Tb, :], op=ALU.is_equal)
        # W_l = ee_l * G_l: per-layer halves, so the weighting pass covers
        # 128 cols total regardless of n_lay (the one-hot O is shared)
        W = wp.tile([P, Tmax, P], dt, tag="W", name="W")
        Wv = W[:, 0:Tb, :].rearrange("p t (l f) -> p t l f", l=n_lay)
        nc.vector.tensor_tensor(
            out=Wv[:],
            in0=G[:, 0:Tb, :].rearrange("p t (l f) -> p t l f", l=n_lay),
            in1=ee[:, 0:Tb, :, :].to_broadcast([P, Tb, n_lay, feat]),
            op=ALU.mult)
        s.update(O=O, W=W, ee=ee)

    def stage3(b):
        """Accumulate numerator/denominator (PE) and finalize the bin."""
        s = st.pop(b)
        O, W, ee, Tb = s["O"], s["W"], s["ee"], s["Tb"]
        # numerator cols [0:feat], denominator col [feat]: one PSUM bank per
        # layer.  num's start=True zeroes the whole bank, so the den matmul
        # always runs start=False and lands on a zeroed column.  All matmuls
        # of a tile share lhsT = the one-hot O_t.
        pn = [pp.tile([P, feat + n_lay if l == 0 else feat], F32,
                      tag=f"pn{l}", name=f"pn{l}")
              for l in range(n_lay)]
        for t in range(Tb):
            for l in range(n_lay):
                nc.tensor.matmul(
                    pn[l][:, 0:feat], lhsT=O[:, t, 0, :],
                    rhs=W[:, t, l * feat:(l + 1) * feat],
                    start=(t == 0), stop=(t == Tb - 1))
            nc.tensor.matmul(
                pn[0][:, feat:feat + n_lay], lhsT=O[:, t, 0, :],
                rhs=ee[:, t, :, 0], start=False, stop=(t == Tb - 1))
        finalize(b, pn)

    # 1-bin software pipeline: keep the TensorEngine fed (its stream is
    # in-order, so bin b+1's xr matmuls must be emitted before bin b's
    # accumulation matmuls to overlap with the DVE/ACT middle stages).
    stage1(0)
    for b in range(cfg.nbins):
        if b + 1 < cfg.nbins:
            stage1(b + 1)
        stage2(b)
        stage3(b)


def _load_consts(nc, cp, names_shapes):
    out = []
    for name, shape, dt in names_shapes:
        dram = nc.dram_tensor(name, shape, dt, kind="ExternalInput")
        sb = cp.tile(shape, dt, name=name + "_sb")
        nc.sync.dma_start(sb[:], dram[:])
        out.append(sb)
    return out


def build_edge1(cfg: Cfg, pr, kpos, dt=F32, nq=1):
    """Edge phase of layer 1 + dense transforms of layers 2/3."""
    nc = _new_nc(cfg, nq)
    hid, out, nlp, nbins = cfg.hid, cfg.out, cfg.nlp, cfg.nbins
    nslots = pr["nslots"]
    Tmax = int(pr["tbin"].max())
    XLchunks = [nc.dram_tensor(f"XL1c{o}", [cfg.chrows, hid], dt,
                               kind="ExternalInput") for o in range(cfg.nchunk)]
    XR = nc.dram_tensor("XR1", [nlp, hid], dt, kind="ExternalInput")
    GIDX = nc.dram_tensor("gidx", [P, nslots // 16], I16, kind="ExternalInput")
    DCOL = nc.dram_tensor("dcol", [P, nslots // 128, 1], dt, kind="ExternalInput")
    OHT = nc.dram_tensor("oht", [P, nslots], dt, kind="ExternalInput")
    XL23 = nc.dram_tensor("XL23", [nlp, P], dt, kind="ExternalOutput")
    XR23 = nc.dram_tensor("XR23", [nlp, P], dt, kind="ExternalOutput")

    with tile.TileContext(nc) as tc:
        with tc.tile_pool(name="const", bufs=1) as cp, \
             tc.tile_pool(name="gath", bufs=3) as gp, \
             tc.tile_pool(name="work", bufs=2) as wp, \
             tc.tile_pool(name="psum", bufs=2, space="PSUM") as pp, \
             tc.tile_pool(name="psfin", bufs=2, space="PSUM") as pf:
            (iotaRep_sb, winv1B_sb, b1B_sb, ident_sb, w23lr_sb,
             b23lr_sb) = _load_consts(nc, cp, [
                 ("iotaRep", [P, Tmax, P], dt),
                 ("winv1B", [P, hid], F32),
                 ("b1B", [P, hid], F32),
                 ("identB", [P, P], F32),
                 ("w23lr", [hid, 2 * P], F32),
                 ("b23lrB", [P, 2 * P], F32)])

            def finalize(b, pn):
                ms = slice(b * P, (b + 1) * P)
                d = wp.tile([P, 1], F32, tag="d", name="d")
                nc.vector.tensor_scalar_add(d[:], pn[0][:, hid:hid + 1], EPS)
                r = wp.tile([P, 1], F32, tag="r", name="r")
                nc.vector.reciprocal(r[:], d[:])
                h = wp.tile([P, hid], F32, tag="h", name="h")
                # h = (num * 1/den) * winv  (unscale att, permuted order)
                nc.vector.scalar_tensor_tensor(
                    out=h[:], in0=pn[0][:, 0:hid], scalar=r[:],
                    in1=winv1B_sb[:], op0=ALU.mult, op1=ALU.mult)
                nc.vector.tensor_tensor(out=h[:], in0=h[:], in1=b1B_sb[:], op=ALU.add)
                nc.scalar.activation(out=h[:], in_=h[:], func=AF.Relu)
                pst = pf.tile([P, P], F32, tag="pst", name="pst")
                nc.tensor.transpose(out=pst[:], in_=h[:], identity=ident_sb[:])
                hT = wp.tile([P, P], F32, tag="hT", name="hT")
                nc.scalar.activation(out=hT[:], in_=pst[:], func=AF.Copy)
                psB = pf.tile([P, 2 * P], F32, tag="psB", name="psB")
                nc.tensor.matmul(psB[:], lhsT=hT[:, 0:hid], rhs=w23lr_sb[:],
                                 start=True, stop=True)
                olr = wp.tile([P, 2 * P], dt, tag="olr", name="olr")
                nc.vector.tensor_tensor(out=olr[:], in0=psB[:], in1=b23lr_sb[:],
                                        op=ALU.add)
                nc.sync.dma_start(XL23[ms, :], olr[:, 0:P])
                nc.sync.dma_start(XR23[ms, :], olr[:, P:2 * P])

            _edge_phase(nc, tc, cfg, pr, (cp, gp, wp, pp),
                        (XLchunks, XR, GIDX, DCOL, OHT),
                        iotaRep_sb, 1, kpos, finalize, dt)
    nc.compile()
    return nc


def build_edge23(cfg: Cfg, pr, kpos, dt=F32, nq=1):
    """Edge phases of layers 2 and 3 (shared gather)."""
    nc = _new_nc(cfg, nq)
    out, nlp, nbins = cfg.out, cfg.nlp, cfg.nbins
    nslots = pr["nslots"]
    Tmax = int(pr["tbin"].max())
    XLchunks = [nc.dram_tensor(f"XL23c{o}", [cfg.chrows, P], dt,
                               kind="ExternalInput") for o in range(cfg.nchunk)]
    XR = nc.dram_tensor("XR23", [nlp, P], dt, kind="ExternalInput")
    GIDX = nc.dram_tensor("gidx", [P, nslots // 16], I16, kind="ExternalInput")
    DCOL = nc.dram_tensor("dcol", [P, nslots // 128, 1], dt, kind="ExternalInput")
    OHT = nc.dram_tensor("oht", [P, nslots], dt, kind="ExternalInput")
    MU = nc.dram_tensor("MU", [nlp, out], F32, kind="ExternalOutput")
    LV = nc.dram_tensor("LV", [nlp, out], F32, kind="ExternalOutput")

    with tile.TileContext(nc) as tc:
        with tc.tile_pool(name="const", bufs=1) as cp, \
             tc.tile_pool(name="gath", bufs=3) as gp, \
             tc.tile_pool(name="work", bufs=2) as wp, \
             tc.tile_pool(name="psum", bufs=2, space="PSUM") as pp:
            (iotaRep_sb, winvmuB_sb, winvlvB_sb, bmu_sb, blv_sb) = _load_consts(
                nc, cp, [
                    ("iotaRep", [P, Tmax, P], dt),
                    ("winvmuB", [P, out], F32),
                    ("winvlvB", [P, out], F32),
                    ("bmuB", [P, out], F32),
                    ("blvB", [P, out], F32)])

            def finalize(b, pn):
                ms = slice(b * P, (b + 1) * P)
                for l, (winv, bias, dest, tg) in enumerate(
                        ((winvmuB_sb, bmu_sb, MU, "mu"),
                         (winvlvB_sb, blv_sb, LV, "lv"))):
                    d = wp.tile([P, 1], F32, tag=f"d{tg}", name="d")
                    nc.vector.tensor_scalar_add(d[:], pn[0][:, out + l:out + l + 1],
                                                EPS)
                    r = wp.tile([P, 1], F32, tag=f"r{tg}", name="r")
                    nc.vector.reciprocal(r[:], d[:])
                    o = wp.tile([P, out], F32, tag=f"o{tg}", name="o")
                    nc.vector.scalar_tensor_tensor(
                        out=o[:], in0=pn[l][:, 0:out], scalar=r[:],
                        in1=winv[:], op0=ALU.mult, op1=ALU.mult)
                    nc.vector.tensor_tensor(out=o[:], in0=o[:], in1=bias[:], op=ALU.add)
                    nc.sync.dma_start(dest[ms, :], o[:])

            _edge_phase(nc, tc, cfg, pr, (cp, gp, wp, pp),
                        (XLchunks, XR, GIDX, DCOL, OHT),
                        iotaRep_sb, 2, kpos, finalize, dt)
    nc.compile()
    return nc


# ----------------------------------------------------------------------------
# Host orchestration
# ----------------------------------------------------------------------------

def _bb(v, rows=P):
    """Broadcast a 1-D row vector to [rows, len] f32."""
    v = np.asarray(v, np.float32).reshape(1, -1)
    return np.ascontiguousarray(np.broadcast_to(v, (rows, v.shape[1])))


def _hw_runner(nc, in_maps, cfg, trace=False):
    from concourse import bass_utils
    r = bass_utils.run_bass_kernel_spmd(
        nc, in_maps, core_ids=list(range(cfg.ncores)), trace=trace)
    return r.results, r.exec_time_ns


class _State:
    """Cached compiled programs + prep, keyed by edge structure."""
    key = None
    progs = None
    prep = None
    fold = None


EDT = BF16 if not int(os.environ.get("GAT_F32", "0")) else F32
NQUEUES = 4


def fold_weights(w):
    """Fold att into the tables: scale columns by att (signed), permute so
    positive-att columns come first in each layer block.  Returns folded
    weights + the inverse data needed at finalize / host postprocess."""
    f = {}
    perms, kpos = {}, {}
    for name, fo in (("sh", HID), ("mu", OUT), ("lv", OUT)):
        a = np.asarray(w[f"{name}_att"], np.float32).reshape(-1)  # [fo]
        pi = np.argsort(a <= 0, kind="stable")  # positive cols first
        perms[name] = pi
        kpos[name] = int((a > 0).sum())
        f[f"{name}_a"] = np.ascontiguousarray(a[pi])  # signed att, permuted
        f[f"{name}_Wl"] = np.ascontiguousarray(
            (np.asarray(w[f"{name}_Wl"], np.float32) * a)[:, pi])
        f[f"{name}_Wr"] = np.ascontiguousarray(
            (np.asarray(w[f"{name}_Wr"], np.float32) * a)[:, pi])
        f[f"{name}_bl"] = (np.asarray(w[f"{name}_bl"], np.float32) * a)[pi]
        f[f"{name}_br"] = (np.asarray(w[f"{name}_br"], np.float32) * a)[pi]
        f[f"{name}_winv"] = 1.0 / f[f"{name}_a"]
        f[f"{name}_b"] = np.asarray(w[f"{name}_b"], np.float32)[pi]
    # layer-1 permutation also permutes h's columns -> permute W23 rows
    pi1 = perms["sh"]
    f["mu_Wl"] = np.ascontiguousarray(f["mu_Wl"][pi1])
    f["mu_Wr"] = np.ascontiguousarray(f["mu_Wr"][pi1])
    f["lv_Wl"] = np.ascontiguousarray(f["lv_Wl"][pi1])
    f["lv_Wr"] = np.ascontiguousarray(f["lv_Wr"][pi1])
    f["perms"] = perms
    f["kpos"] = kpos
    return f


def build_progs(cfg, pr, fold, dt=None, nq=None):
    dt = EDT if dt is None else dt
    nq = NQUEUES if nq is None else nq
    kp = fold["kpos"]
    return dict(
        dense1=build_dense1(cfg, dt),
        edge1=build_edge1(cfg, pr, [kp["sh"]], dt, nq),
        edge23=build_edge23(cfg, pr, [kp["mu"], kp["lv"]], dt, nq),
    )


def forward(cfg, x, ei_unused, w, f, pr, progs, runner, dt=None):
    dt = EDT if dt is None else dt
    ndt = mybir.dt.np(dt)
    perm = pr["perm"]                    # [ncores, nlp] node ids or -1
    Tmax = int(pr["tbin"].max())
    profile = {}
    dcol_in = [np.ascontiguousarray(pr["dstcol"][c][:, :, None].astype(ndt))
               for c in range(cfg.ncores)]
    oht_in = [np.ascontiguousarray(pr["onehotT"][c].astype(ndt))
              for c in range(cfg.ncores)]

    hid, out, nlp, ntab = cfg.hid, cfg.out, cfg.nlp, cfg.ntab

    # ---- launch A: dense1 (att-scaled, sign-permuted weights) -------------
    in_maps = []
    for c in range(cfg.ncores):
        xs = np.zeros((nlp, cfg.fin), np.float32)
        sel = perm[c] >= 0
        xs[sel] = x[perm[c][sel]]
        in_maps.append(dict(
            xT=np.ascontiguousarray(xs.T), wl=f["sh_Wl"], wr=f["sh_Wr"],
            blB=_bb(f["sh_bl"]), brB=_bb(f["sh_br"])))
    rA, profile["A"] = runner(progs["dense1"], in_maps, cfg)
    XL1full = np.concatenate([rA[c]["XL1"] for c in range(cfg.ncores)])
    XL1ch = {f"XL1c{o}": np.ascontiguousarray(
        XL1full[o * cfg.chrows:(o + 1) * cfg.chrows])
        for o in range(cfg.nchunk)}
    XR1 = [rA[c]["XR1"] for c in range(cfg.ncores)]

    # ---- launch B: edge1 + dense23 ----------------------------------------
    iotaRep = np.ascontiguousarray(np.broadcast_to(
        np.arange(P, dtype=np.float32), (P, Tmax, P))).astype(ndt)
    w23l = np.concatenate([f["mu_Wl"], f["lv_Wl"]], axis=1)
    w23r = np.concatenate([f["mu_Wr"], f["lv_Wr"]], axis=1)
    b23l = np.concatenate([f["mu_bl"], f["lv_bl"]])
    b23r = np.concatenate([f["mu_br"], f["lv_br"]])
    ident = np.eye(P, dtype=np.float32)
    in_maps = []
    for c in range(cfg.ncores):
        in_maps.append(dict(
            XR1=XR1[c], **XL1ch,
            gidx=pr["gidx16"][c], dcol=dcol_in[c], oht=oht_in[c],
            iotaRep=iotaRep, winv1B=_bb(f["sh_winv"]), b1B=_bb(f["sh_b"]),
            identB=ident,
            w23lr=np.ascontiguousarray(np.concatenate([w23l, w23r], axis=1)),
            b23lrB=_bb(np.concatenate([b23l, b23r]))))
    rB, profile["B"] = runner(progs["edge1"], in_maps, cfg)
    XL23full = np.concatenate([rB[c]["XL23"] for c in range(cfg.ncores)])
    XL23ch = {f"XL23c{o}": np.ascontiguousarray(
        XL23full[o * cfg.chrows:(o + 1) * cfg.chrows])
        for o in range(cfg.nchunk)}
    XR23 = [rB[c]["XR23"] for c in range(cfg.ncores)]

    # ---- launch C: edge23 --------------------------------------------------
    in_maps = []
    for c in range(cfg.ncores):
        in_maps.append(dict(
            XR23=XR23[c], **XL23ch,
            gidx=pr["gidx16"][c], dcol=dcol_in[c], oht=oht_in[c],
            iotaRep=iotaRep,
            winvmuB=_bb(f["mu_winv"]), winvlvB=_bb(f["lv_winv"]),
            bmuB=_bb(f["mu_b"]), blvB=_bb(f["lv_b"])))
    rC, profile["C"] = runner(progs["edge23"], in_maps, cfg)

    MU = np.concatenate([rC[c]["MU"] for c in range(cfg.ncores)])
    LV = np.concatenate([rC[c]["LV"] for c in range(cfg.ncores)])
    # undo the sign permutation of the output columns
    mu_u = np.empty_like(MU)
    mu_u[:, f["perms"]["mu"]] = MU
    lv_u = np.empty_like(LV)
    lv_u[:, f["perms"]["lv"]] = LV
    mu = mu_u[pr["slot_global"]]
    lv = lv_u[pr["slot_global"]]
    return (mu, lv), profile


def kernel(**inputs):
    cfg = Cfg()
    x = np.asarray(inputs["x"], np.float32)
    ei = np.asarray(inputs["edge_index"]).astype(np.int64)
    w = {k: np.asarray(v, np.float32) for k, v in inputs.items()
         if k not in ("x", "edge_index")}

    fold = fold_weights(w)
    key = (hash(ei.tobytes()), tuple(sorted(fold["kpos"].items())))
    if _State.key != key:
        pr = prep_graph(cfg, ei)
        _State.prep = pr
        _State.progs = build_progs(cfg, pr, fold)
        _State.key = key
    _State.fold = fold

    trace = bool(int(os.environ.get("GAT_TRACE", "0")))
    runner = functools.partial(_hw_runner, trace=trace)
    (mu, lv), profile = forward(cfg, x, ei, w, _State.fold, _State.prep,
                                _State.progs, runner)
    kernel._last_profile = profile
    return (mu, lv)


kernel._last_profile = None
